# revision 17
# baseline (speedup 1.0000x reference)
"""CPC model (conv encoder + GRU + InfoNCE loss) on 8 TRN2 NeuronCores.

Strategy:
 - Data-parallel over batch: each core owns 8 of 64 sequences (72 images).
 - Conv encoder runs per image-pair as bf16 matmuls (f32 PSUM accum):
     conv1 5x5s2 via host im2col (K=75 padded to 128), resblock 3x3 via
     9-tap shifted matmuls over a zero-padded [18,18] activation.
 - Timestep rows processed in order [4..8, 0..3]: the ztk rows finish early
   so the AllGather overlaps the remaining conv; GRU step t is emitted right
   after row t so it hides under the next row's conv.
 - PSUM->SBUF drains are split across Scalar/Vector/GpSimd engines so the
   in-order scalar queue never head-of-line-blocks the PE; weight DMAs are
   triggered from the (otherwise idle) gpsimd queue for the same reason.
 - Only one activation table (sigmoid/tanh/relu) is used on device; the
   double-exp log-softmax runs on the host in float64 from the raw [8, 320]
   per-core score block (the softmax is tiny; argmax/loss math is exact).
"""
import os
import sys

import numpy as np
import ml_dtypes

for _p in ("/opt/trn_rl_repo", "/root/.axon_site/_ro/trn_rl_repo"):
    if os.path.isdir(_p) and _p not in sys.path:
        sys.path.insert(0, _p)

import concourse.bacc as bacc  # noqa: E402
import concourse.bass as bass  # noqa: E402
import concourse.mybir as mybir  # noqa: E402
import concourse.tile as tile  # noqa: E402
from concourse.bass_utils import run_bass_kernel_spmd  # noqa: E402

F32 = np.float32
BF16 = ml_dtypes.bfloat16
E4M3 = ml_dtypes.float8_e4m3
DT = mybir.dt

B, T, C = 64, 9, 3
DIM, HALF, HID, R, K = 512, 256, 256, 2, 5
TCTX = 4
NCORES = 8
NB = B // NCORES           # 8
NIMG = NB * T              # 72
NPIX = 256                 # 16*16
NPAIR = NIMG // 2          # 36
ROWS = [4, 5, 6, 7, 8, 0, 1, 2, 3]
ALU = mybir.AluOpType
ACTF = mybir.ActivationFunctionType


def build_kernel():
    nc = bacc.Bacc("TRN2", target_bir_lowering=False, debug=False,
                   num_devices=NCORES)

    def din(name, shape, dt):
        return nc.dram_tensor(name, shape, dt, kind="ExternalInput")

    xcol_d = din("xcol", [NPAIR, 128, 512], DT.bfloat16)
    w1T_d = din("w1T", [128, DIM], DT.bfloat16)
    r1T_d = din("r1T", [128, R, 4, HALF], DT.bfloat16)
    # fp8 (e4m3) copies of resblock-0's 1x1 conv weights, x64 scaled,
    # packed for DoubleRow: [part, cpair, kslot, mchunk, 128]
    r1T8_d = din("r1T8", [128, 2, 2, 2, 128], DT.float8e4)
    r3T8_d = din("r3T8", [128, 2, 4, 128], DT.float8e4)
    w2T_d = din("w2T", [128, R, 9, 2, HALF], DT.bfloat16)
    r3T_d = din("r3T", [128, R, 2, DIM], DT.bfloat16)
    encb_d = din("encb", [128, 4], DT.float32)
    b1_d = din("b1", [128, R, 2], DT.float32)
    b2_d = din("b2", [128, R, 2], DT.float32)
    b3_d = din("b3", [128, R, 4], DT.float32)
    gihT_d = din("gihT", [128, 4, 3 * HID], DT.bfloat16)
    ghhT_d = din("ghhT", [128, 2, 3 * HID], DT.bfloat16)
    gbih_d = din("gbih", [NB, 3 * HID], DT.float32)   # host-broadcast over batch
    gbhh_d = din("gbhh", [NB, 3 * HID], DT.float32)
    gbc_d = din("gbc", [NB, 2 * HID], DT.float32)     # (b_ih + b_hh)[: 512]
    wkT2_d = din("wkT2", [128, 4, K, HID], DT.bfloat16)
    wkbrep_d = din("wkbrep", [128, 4, K, NB], DT.bfloat16)
    ident_d = din("ident8", [NB, NB], DT.float32)

    out_d = nc.dram_tensor("out", [NB, K * B], DT.float32, kind="ExternalOutput")

    # collective bounce buffers (internal DRAM, partition-major for cheap DMA)
    zin_b = nc.dram_tensor("zin_b", [128, 4, K * NB], DT.bfloat16)
    zout_b = nc.dram_tensor("zout_b", [NCORES, 128, 4, K, NB], DT.bfloat16,
                            addr_space="Shared")

    from contextlib import ExitStack
    with tile.TileContext(nc) as tc, ExitStack() as stack:
        wp = stack.enter_context(tc.tile_pool(name="weights", bufs=1))
        persist = stack.enter_context(tc.tile_pool(name="persist", bufs=1))
        y1p_pool = stack.enter_context(tc.tile_pool(name="y1p", bufs=4))
        xcp = stack.enter_context(tc.tile_pool(name="xc", bufs=6))
        hp = stack.enter_context(tc.tile_pool(name="h", bufs=8))
        y2p = stack.enter_context(tc.tile_pool(name="y2", bufs=4))
        psp = stack.enter_context(tc.tile_pool(name="psum", bufs=8, space="PSUM"))
        sp = stack.enter_context(tc.tile_pool(name="small", bufs=2))

        # Preload the sigmoid/tanh/relu activation table before any relu so
        # the scalar engine never swaps tables mid-kernel (a swap is 1.28us
        # on the in-order queue and stalls the PE behind PSUM drains).
        junk = persist.tile([NB, 1], DT.float32, tag="junk")
        nc.vector.memset(junk[:], 0.0)
        nc.scalar.activation(junk[:], junk[:], ACTF.Sigmoid)

        # PE warmup: ~30 dummy matmuls on a zero tile keep the PE busy during
        # the initial input/weight DMA wait, so the HAM clock-gate reaches
        # 8/8 before the first real conv matmul (instead of ramping through
        # the first ~3.4us of real work at 1.2 GHz).
        warm = persist.tile([128, 128], DT.bfloat16, tag="warm")
        nc.vector.memset(warm[:], 0.0)
        wps = psp.tile([128, 128], DT.float32, tag="ps", name="warmps")
        for _ in range(30):
            nc.tensor.matmul(wps[:], warm[:], warm[:], start=True, stop=True)

        def wtile(dram, shape, dt, split_dim1=False):
            t = wp.tile(shape, dt, tag=dram.name, name=f"w_{dram.name}")
            if split_dim1:
                for i in range(shape[1]):
                    nc.gpsimd.dma_start(t[:, i], dram[:, i])
            else:
                nc.gpsimd.dma_start(t[:], dram[:])
            return t

        # conv weights first (needed by pair 0); w1T split in four so the
        # first conv1 LDWEIGHTS chunk lands sooner; w2T split per-resblock
        w1T = wp.tile([128, DIM], DT.bfloat16, tag="w1T", name="w_w1T")
        for mc in range(4):
            nc.gpsimd.dma_start(w1T[:, mc * 128:(mc + 1) * 128],
                                w1T_d[:, mc * 128:(mc + 1) * 128])
        encb = wtile(encb_d, [128, 4], DT.float32)
        r1T = wtile(r1T_d, [128, R, 4, HALF], DT.bfloat16)
        r1T8 = wtile(r1T8_d, [128, 2, 2, 2, 128], DT.float8e4)
        r3T8 = wtile(r3T8_d, [128, 2, 4, 128], DT.float8e4)
        b1 = wtile(b1_d, [128, R, 2], DT.float32)
        w2T = wtile(w2T_d, [128, R, 9, 2, HALF], DT.bfloat16, split_dim1=True)
        b2 = wtile(b2_d, [128, R, 2], DT.float32)
        r3T = wtile(r3T_d, [128, R, 2, DIM], DT.bfloat16)
        b3 = wtile(b3_d, [128, R, 4], DT.float32)
        # gru / loss weights (needed only after row t=0)
        gihT = wtile(gihT_d, [128, 4, 3 * HID], DT.bfloat16)
        ghhT = wtile(ghhT_d, [128, 2, 3 * HID], DT.bfloat16)
        gbih = wtile(gbih_d, [NB, 3 * HID], DT.float32)
        gbhh = wtile(gbhh_d, [NB, 3 * HID], DT.float32)
        gbc = wtile(gbc_d, [NB, 2 * HID], DT.float32)
        wkT2 = wtile(wkT2_d, [128, 4, K, HID], DT.bfloat16)
        wkbrep = wtile(wkbrep_d, [128, 4, K, NB], DT.bfloat16)
        ident = wtile(ident_d, [NB, NB], DT.float32)

        zbuf = persist.tile([128, 4, NIMG], DT.bfloat16, tag="zbuf")
        hT = persist.tile([128, 2, NB], DT.bfloat16, tag="hT")
        y40 = persist.tile([NB, K * B], DT.float32, tag="y40")
        # one padded activation tile per (pair parity, resblock): breaks the
        # write-after-read chain between consecutive pairs / resblocks
        y1pads = [y1p_pool.tile([128, 2, 2, 18, 18], DT.bfloat16, tag="y1pad",
                                name=f"y1pad{i}") for i in range(4)]
        for ypad in y1pads:
            nc.vector.memset(ypad[:], 0.0)

        # -------- conv encoder: two pairs in lockstep (software pipeline) --
        # Each stage of pair A is followed by the same stage of pair B, so
        # A's PSUM->SBUF drains always have a sibling stage of independent
        # matmuls behind them in the in-order PE queue (no head-of-line
        # stalls on ACT/DVE latency).
        conv1_seq = [0]

        def emit_conv1(p, st):
            xc = xcp.tile([128, 512], DT.bfloat16, tag="xc", name=f"xc{p}")
            seq = conv1_seq[0]
            conv1_seq[0] += 1
            if seq < 4:
                # first-slot pairs gate the kernel head: split each image-pair
                # DMA across two trigger queues (two DMA rings in parallel)
                nc.sync.dma_start(xc[:, :256], xcol_d[p][:, :256])
                nc.scalar.dma_start(xc[:, 256:], xcol_d[p][:, 256:])
            elif seq % 2 == 0:
                nc.sync.dma_start(xc[:], xcol_d[p])
            else:
                nc.scalar.dma_start(xc[:], xcol_d[p])
            h = hp.tile([128, 4, 512], DT.bfloat16, tag="h", name=f"h{p}")
            h8 = hp.tile([128, 4, 512], DT.float8e4, tag="h8", name=f"h8{p}")
            for m in range(4):
                ps = psp.tile([128, 512], DT.float32, tag="ps", name=f"c1ps{p}{m}")
                nc.tensor.matmul(ps[:], w1T[:, m * 128:(m + 1) * 128], xc[:],
                                 start=True, stop=True)
                nc.scalar.activation(h[:, m], ps[:], ACTF.Relu,
                                     bias=encb[:, m:m + 1])
                # fp8 copy (Pool engine) feeds resblock-0's DoubleRow rb_a
                nc.gpsimd.tensor_copy(h8[:, m], h[:, m])
            st['h'] = h
            st['h8'] = h8

        def emit_rb_a(p, r, st):
            y1p = y1pads[(p % 2) * 2 + r]
            h = st['h']
            ps_a = [psp.tile([128, 512], DT.float32, tag="ps",
                             name=f"a_ps{p}{r}{m}") for m in range(2)]
            if r == 0:
                # fp8 DoubleRow: 2 k-chunk-pairs x 2 m -> 4 MMs (vs 8 bf16);
                # weights x64 on host, undone by the drain's 2^-6 scale
                h8 = st['h8']
                for m in range(2):
                    for j in range(2):
                        nc.tensor.matmul(
                            ps_a[m][:], r1T8[:, j, :, m, :],
                            h8[:, 2 * j:2 * j + 2, :],
                            start=(j == 0), stop=(j == 1),
                            perf_mode=mybir.MatmulPerfMode.DoubleRow)
            else:
                for m in range(2):
                    for c in range(4):
                        nc.tensor.matmul(ps_a[m][:],
                                         r1T[:, r, c, m * 128:(m + 1) * 128],
                                         h[:, c], start=(c == 0), stop=(c == 3))
            for m in range(2):
                nc.scalar.activation(y1p[:, m, :, 1:17, 1:17],
                                     ps_a[m][:].rearrange("p (i r c) -> p i r c",
                                                          i=2, r=16),
                                     ACTF.Relu, bias=b1[:, r, m:m + 1],
                                     scale=(0.015625 if r == 0 else 1.0))
            st['y1p'] = y1p
            st['y2'] = y2p.tile([128, 2, 512], DT.bfloat16, tag="y2",
                                name=f"y2_{p}{r}")
            if r == 0:
                st['y28'] = y2p.tile([128, 2, 512], DT.float8e4, tag="y28",
                                     name=f"y28_{p}{r}")

        # 3x3 taps ordered center-first: the center tap covers the full 16x16
        # output (start=True clears the bank); edge taps touch only the output
        # rows/cols whose shifted input window stays inside the real image, so
        # the MACs that would multiply pad zeros are skipped entirely
        # (bit-identical: dropping x+0.0 accumulations).
        TAPS = []
        for tap in [4, 0, 1, 2, 3, 5, 6, 7, 8]:
            ky, kx = divmod(tap, 3)
            y0 = 1 if ky == 0 else 0
            x0 = 1 if kx == 0 else 0
            th = 15 if ky != 1 else 16
            tw = 15 if kx != 1 else 16
            TAPS.append((tap, ky, kx, y0, x0, th, tw))

        def emit_rb_b(p, r, m, st):
            y1p, y2 = st['y1p'], st['y2']
            ps = psp.tile([128, 512], DT.float32, tag="ps",
                          name=f"b_ps{p}{r}{m}")
            psv = ps[:].rearrange("p (i r c) -> p i r c", i=2, r=16)
            i_mm = 0
            for c in range(2):
                for tap, ky, kx, y0, x0, th, tw in TAPS:
                    rhs = y1p[:, c, :, ky + y0:ky + y0 + th,
                              kx + x0:kx + x0 + tw]
                    nc.tensor.matmul(
                        psv[:, :, y0:y0 + th, x0:x0 + tw],
                        w2T[:, r, tap, c, m * 128:(m + 1) * 128],
                        rhs, start=(i_mm == 0), stop=(i_mm == 17))
                    i_mm += 1
            nc.scalar.activation(y2[:, m], ps[:], ACTF.Relu,
                                 bias=b2[:, r, m:m + 1])
            if r == 0:
                nc.gpsimd.tensor_copy(st['y28'][:, m], y2[:, m])

        def emit_rb_c(p, r, st, skip_relu=False):
            h, y2 = st['h'], st['y2']
            hn = hp.tile([128, 4, 512], DT.bfloat16, tag="h", name=f"hn{p}{r}")
            ps3 = [psp.tile([128, 512], DT.float32, tag="ps",
                            name=f"c_ps{p}{r}{m}") for m in range(4)]
            if r == 0:
                # fp8 DoubleRow: one k-pair covers the full 256 contraction
                y28 = st['y28']
                for m in range(4):
                    nc.tensor.matmul(
                        ps3[m][:], r3T8[:, :, m, :], y28[:],
                        start=True, stop=True,
                        perf_mode=mybir.MatmulPerfMode.DoubleRow)
            else:
                # m0/m1 groups complete early so their DVE residual chain
                # overlaps the m2/m3 matmuls
                for m, c in ((0, 0), (1, 0), (0, 1), (1, 1),
                             (2, 0), (3, 0), (2, 1), (3, 1)):
                    nc.tensor.matmul(ps3[m][:],
                                     r3T[:, r, c, m * 128:(m + 1) * 128],
                                     y2[:, c], start=(c == 0), stop=(c == 1))
            for m in range(4):
                if r == 0:
                    # hn = psum * 2^-6 + h (weights were x64 on host)
                    nc.vector.scalar_tensor_tensor(
                        hn[:, m], ps3[m][:], 0.015625, h[:, m],
                        ALU.mult, ALU.add)
                else:
                    nc.vector.tensor_add(hn[:, m], ps3[m][:], h[:, m])
                if skip_relu:
                    continue    # relu fused into the zt accum ACT
                # alternate relu between ACT and DVE: the residual drain
                # is throughput-bound at the rb boundary, so split it
                if m % 2 == 0:
                    nc.scalar.activation(hn[:, m], hn[:, m], ACTF.Relu,
                                         bias=b3[:, r, m:m + 1])
                else:
                    nc.vector.tensor_scalar(hn[:, m], hn[:, m],
                                            b3[:, r, m:m + 1], 0.0,
                                            ALU.add, ALU.max)
            st['h'] = hn

        def emit_zt(p, st):
            t_idx, j_idx = divmod(p, 4)
            col = t_idx * 8 + 2 * j_idx
            h = st['h']
            zt = sp.tile([128, 4, 2], DT.float32, tag="zt", name=f"zt{p}")
            # per-chunk reduce so the last pair's GRU inputs materialize
            # incrementally (chunk c gates only the c-th gi matmul)
            for c in range(4):
                nc.vector.tensor_reduce(
                    zt[:, c], h[:, c].rearrange("p (i x) -> p i x", i=2),
                    mybir.AxisListType.X, ALU.add)
                nc.vector.tensor_scalar_mul(zbuf[:, c, col:col + 2], zt[:, c],
                                            1.0 / NPIX)

        def emit_zt_fused(pA, stA, pB, stB):
            # last-slot variant: relu+bias+spatial-sum fused into one ACT op
            # per (chunk, image) via accum_out, pairs interleaved per chunk,
            # so the final GRU step's zbuf row materializes ~3us sooner
            acc = sp.tile([128, 16], DT.float32, tag="ztacc")
            for c in range(4):
                for i, (p, st) in enumerate(((pA, stA), (pB, stB))):
                    col = (p // 4) * 8 + 2 * (p % 4)
                    h = st['h']
                    for img in range(2):
                        a = acc[:, c * 4 + i * 2 + img:c * 4 + i * 2 + img + 1]
                        nc.scalar.activation(
                            h[:, c, img * NPIX:(img + 1) * NPIX],
                            h[:, c, img * NPIX:(img + 1) * NPIX],
                            ACTF.Relu, bias=b3[:, 1, c:c + 1],
                            accum_out=a)
                        nc.vector.tensor_scalar_mul(
                            zbuf[:, c, col + img:col + img + 1],
                            a, 1.0 / NPIX)

        def emit_slot(pA, pB, stA, stB, nxt=None, hook=None, last=False,
                      nxt_early=False):
            # stA/stB carry this slot's conv1 outputs (prefetched by the
            # previous slot); nxt = (pA', pB', stA', stB') whose conv1 stage
            # is emitted before this slot's final rb-c so the slot-boundary
            # drains always have independent matmuls behind them.
            # nxt_early (first slot only): emit it right after the rb-a
            # stage instead, to cover the cold-start drain stalls.
            for r in range(R):
                emit_rb_a(pA, r, stA)
                emit_rb_a(pB, r, stB)
                if r == 0 and nxt_early and nxt is not None:
                    emit_conv1(nxt[0], nxt[2])
                    emit_conv1(nxt[1], nxt[3])
                emit_rb_b(pA, r, 0, stA)
                emit_rb_b(pB, r, 0, stB)
                if r == 0 and hook is not None:
                    # GRU work for the previous row: emitted ~9us into the
                    # slot so the previous row's zt DVE backlog has drained
                    hook()
                emit_rb_b(pA, r, 1, stA)
                emit_rb_b(pB, r, 1, stB)
                if r == R - 1 and nxt is not None and not nxt_early:
                    emit_conv1(nxt[0], nxt[2])
                    emit_conv1(nxt[1], nxt[3])
                emit_rb_c(pA, r, stA, skip_relu=(last and r == R - 1))
                emit_rb_c(pB, r, stB, skip_relu=(last and r == R - 1))
            if last:
                emit_zt_fused(pA, stA, pB, stB)
            else:
                emit_zt(pA, stA)
                emit_zt(pB, stB)

        # ---------------- GRU step (emitted after row t) ----------------
        gru_state = {'h': None}

        def emit_gru_mms(t):
            gi_rz = psp.tile([NB, 2 * HID], DT.float32, tag="ps", name=f"girz{t}")
            gi_n = psp.tile([NB, HID], DT.float32, tag="ps", name=f"gin{t}")
            for c in range(4):
                nc.tensor.matmul(gi_rz[:], zbuf[:, c, t * 8:(t + 1) * 8],
                                 gihT[:, c, :2 * HID],
                                 start=(c == 0), stop=(c == 3))
            for c in range(4):
                nc.tensor.matmul(gi_n[:], zbuf[:, c, t * 8:(t + 1) * 8],
                                 gihT[:, c, 2 * HID:],
                                 start=(c == 0), stop=(c == 3))
            gh_rz = gh_n = None
            if t > 0:
                gh_rz = psp.tile([NB, 2 * HID], DT.float32, tag="ps",
                                 name=f"ghrz{t}")
                gh_n = psp.tile([NB, HID], DT.float32, tag="ps", name=f"ghn{t}")
                for c in range(2):
                    nc.tensor.matmul(gh_rz[:], hT[:, c, :], ghhT[:, c, :2 * HID],
                                     start=(c == 0), stop=(c == 1))
                for c in range(2):
                    nc.tensor.matmul(gh_n[:], hT[:, c, :], ghhT[:, c, 2 * HID:],
                                     start=(c == 0), stop=(c == 1))
            return gi_rz, gi_n, gh_rz, gh_n

        def emit_gru_chain(t, gi_rz, gi_n, gh_rz, gh_n, split_final=False):
            rz = sp.tile([NB, 2 * HID], DT.float32, tag="rz", name=f"rz{t}")
            ng = sp.tile([NB, HID], DT.float32, tag="ng", name=f"ng{t}")
            tmp = sp.tile([NB, HID], DT.float32, tag="gtmp", name=f"gtmp{t}")
            # r,z = sigmoid(gi_rz + gh_rz + (b_ih + b_hh)[:512])
            nc.vector.tensor_add(rz[:], gi_rz[:], gbc[:])
            if t > 0:
                nc.vector.tensor_add(rz[:], rz[:], gh_rz[:])
            nc.scalar.activation(rz[:], rz[:], ACTF.Sigmoid)
            # n = tanh(gi_n + b_ih_n + r * (gh_n + b_hh_n))
            if t > 0:
                nc.vector.tensor_add(tmp[:], gh_n[:], gbhh[:, 2 * HID:])
            else:
                nc.vector.tensor_copy(tmp[:], gbhh[:, 2 * HID:])
            nc.vector.tensor_mul(tmp[:], tmp[:], rz[:, :HID])
            nc.vector.tensor_add(ng[:], gi_n[:], gbih[:, 2 * HID:])
            nc.vector.tensor_add(ng[:], ng[:], tmp[:])
            nc.scalar.activation(ng[:], ng[:], ACTF.Tanh)
            # h = (1-z)*n + z*h_prev = n + z*(h_prev - n)
            h_new = sp.tile([NB, HID], DT.float32, tag=f"hstep{t}",
                            name=f"hnew{t}")
            if split_final:
                # per-128-chunk so each hT transpose starts as soon as its
                # half of h is ready (shortens the endgame critical path)
                h_prev = gru_state['h']
                for c in range(2):
                    cs = slice(c * 128, (c + 1) * 128)
                    nc.vector.tensor_sub(tmp[:, cs], h_prev[:, cs], ng[:, cs])
                    nc.vector.tensor_mul(tmp[:, cs],
                                         rz[:, HID + c * 128:HID + (c + 1) * 128],
                                         tmp[:, cs])
                    nc.vector.tensor_add(h_new[:, cs], ng[:, cs], tmp[:, cs])
                    pt = psp.tile([128, NB], DT.float32, tag="ps",
                                  name=f"ptf{c}")
                    nc.tensor.transpose(pt[:], h_new[:, cs], ident[:])
                    nc.vector.tensor_copy(hT[:, c, :], pt[:])
            elif t == 0:
                nc.vector.tensor_mul(tmp[:], rz[:, HID:], ng[:])
                nc.vector.tensor_sub(h_new[:], ng[:], tmp[:])
            else:
                nc.vector.tensor_sub(tmp[:], gru_state['h'][:], ng[:])
                nc.vector.tensor_mul(tmp[:], rz[:, HID:], tmp[:])
                nc.vector.tensor_add(h_new[:], ng[:], tmp[:])
            gru_state['h'] = h_new

        def emit_gru_step(t):
            emit_gru_chain(t, *emit_gru_mms(t))

        def emit_transposes(t):
            h_new = gru_state['h']
            for c in range(2):
                pt = psp.tile([128, NB], DT.float32, tag="ps", name=f"pt{t}{c}")
                nc.tensor.transpose(pt[:], h_new[:, c * 128:(c + 1) * 128],
                                    ident[:])
                nc.vector.tensor_copy(hT[:, c, :], pt[:])

        # -------- emit: conv rows with GRU / collective interleaved --------
        # hooks[s] = ops to emit inside slot s of the NEXT row, so GRU work
        # hides under conv instead of stalling the PE queue.
        hooks = {0: [], 1: []}
        # gathered z, contiguous b innermost: ztk[p, dc, k, b]
        ztk = persist.tile([128, 4, K, B], DT.bfloat16, tag="ztk")
        GT = persist.tile([128, 2, K, B], DT.bfloat16, tag="GT")

        def emit_G():
            # G[b,k,h] = sum_d ztk[b,k,d] * wk_w[k,d,h], computed mid-conv
            # once the AllGather lands, so the endgame scores need only the
            # tiny ct x G matmuls after the final GRU step
            for k in range(K):
                for hc in range(2):
                    psg = psp.tile([128, B], DT.float32, tag="ps",
                                   name=f"g{k}{hc}")
                    for dc in range(4):
                        nc.tensor.matmul(
                            psg[:], wkT2[:, dc, k, hc * 128:(hc + 1) * 128],
                            ztk[:, dc, k, :],
                            start=(dc == 0), stop=(dc == 3))
                    nc.vector.tensor_copy(GT[:, hc, k, :], psg[:])

        gru3_prep = {}

        def emit_gru3_prep():
            # step-3 hidden-state matmuls + bias merges, emitted a row early
            # (hT(2) is ready) so only the gi-dependent ops remain at the end
            gh_rz = psp.tile([NB, 2 * HID], DT.float32, tag="ps", name="ghrz3")
            gh_n = psp.tile([NB, HID], DT.float32, tag="ps", name="ghn3")
            for c in range(2):
                nc.tensor.matmul(gh_rz[:], hT[:, c, :], ghhT[:, c, :2 * HID],
                                 start=(c == 0), stop=(c == 1))
            for c in range(2):
                nc.tensor.matmul(gh_n[:], hT[:, c, :], ghhT[:, c, 2 * HID:],
                                 start=(c == 0), stop=(c == 1))
            rzh = sp.tile([NB, 2 * HID], DT.float32, tag="rzh")
            tmp0 = sp.tile([NB, HID], DT.float32, tag="tmp0")
            nc.vector.tensor_add(rzh[:], gh_rz[:], gbc[:])
            nc.vector.tensor_add(tmp0[:], gh_n[:], gbhh[:, 2 * HID:])
            gru3_prep['rzh'] = rzh
            gru3_prep['tmp0'] = tmp0

        def run_hooks(s):
            for fn in hooks[s]:
                fn()
            hooks[s] = []

        # flatten slots so each slot can prefetch the next slot's conv1
        slot_list = []
        for t in ROWS:
            slot_list.append((t * 4 + 0, t * 4 + 1, 0, False))
            slot_list.append((t * 4 + 2, t * 4 + 3, 1, t == 3))
        slot_states = [({}, {}) for _ in slot_list]
        emit_conv1(slot_list[0][0], slot_states[0][0])
        emit_conv1(slot_list[0][1], slot_states[0][1])

        def emit_row_slots(t):
            for s in (0, 1):
                idx = ROWS.index(t) * 2 + s
                pA, pB, hs, last = slot_list[idx]
                nxt = None
                if idx + 1 < len(slot_list):
                    n = slot_list[idx + 1]
                    nxt = (n[0], n[1],
                           slot_states[idx + 1][0], slot_states[idx + 1][1])
                emit_slot(pA, pB, slot_states[idx][0], slot_states[idx][1],
                          nxt=nxt,
                          hook=(lambda s=hs: run_hooks(s)) if hooks[hs] else None,
                          last=last, nxt_early=(idx == 0))

        for t in ROWS:
            emit_row_slots(t)
            if t == 8:
                # ztk rows complete -> AllGather (gpsimd queue, overlaps conv)
                nc.gpsimd.dma_start(zin_b.ap(), zbuf[:, :, TCTX * 8:])
                nc.gpsimd.collective_compute(
                    "AllGather", ALU.bypass,
                    replica_groups=[list(range(NCORES))],
                    ins=[zin_b.ap().opt()], outs=[zout_b.ap().opt()])
                for core in range(NCORES):
                    nc.gpsimd.dma_start(
                        ztk[:, :, :, core * NB:(core + 1) * NB], zout_b[core])
            if t < 3:
                hooks[0].append(lambda t=t: emit_gru_step(t))
                hooks[1].append(lambda t=t: emit_transposes(t))
                if t == 1:
                    # gathered ztk lands mid row 1 -> compute G during row 2
                    hooks[0].append(emit_G)
                if t == 2:
                    hooks[1].append(emit_gru3_prep)

        # ---------------- endgame: GRU step 3 + raw scores ----------------
        # (host does the double-exp log-softmax in float64 from raw scores)
        gi_rz3 = psp.tile([NB, 2 * HID], DT.float32, tag="ps", name="girz3")
        gi_n3 = psp.tile([NB, HID], DT.float32, tag="ps", name="gin3")
        for c in range(4):
            nc.tensor.matmul(gi_rz3[:], zbuf[:, c, 24:32],
                             gihT[:, c, :2 * HID], start=(c == 0), stop=(c == 3))
        for c in range(4):
            nc.tensor.matmul(gi_n3[:], zbuf[:, c, 24:32],
                             gihT[:, c, 2 * HID:], start=(c == 0), stop=(c == 3))
        # single-bank score accumulator: bias + ct matmuls write disjoint
        # 64-col slices of one [8, 320] PSUM tile as one accumulation group
        # (per-element has_written bits make the slices independent)
        psk = psp.tile([NB, K * B], DT.float32, tag="ps", name="sck")
        for k in range(K):
            for dc in range(4):
                nc.tensor.matmul(psk[:, k * B:(k + 1) * B],
                                 wkbrep[:, dc, k, :], ztk[:, dc, k, :],
                                 start=(k == 0 and dc == 0), stop=False)
        # GRU-3 chain with the gh/bias parts pre-merged (rzh, tmp0); r/z and
        # tanh/finale split so each consumer unblocks as early as possible
        rzh, tmp0 = gru3_prep['rzh'], gru3_prep['tmp0']
        rz = sp.tile([NB, 2 * HID], DT.float32, tag="rz", name="rz3")
        ng = sp.tile([NB, HID], DT.float32, tag="ng", name="ng3")
        tmpf = sp.tile([NB, HID], DT.float32, tag="gtmp", name="gtmp3")
        nc.vector.tensor_add(rz[:, :HID], gi_rz3[:, :HID], rzh[:, :HID])
        nc.vector.tensor_add(ng[:], gi_n3[:], gbih[:, 2 * HID:])
        nc.vector.tensor_add(rz[:, HID:], gi_rz3[:, HID:], rzh[:, HID:])
        nc.scalar.activation(rz[:, :HID], rz[:, :HID], ACTF.Sigmoid)
        nc.scalar.activation(rz[:, HID:], rz[:, HID:], ACTF.Sigmoid)
        nc.vector.tensor_mul(tmpf[:], tmp0[:], rz[:, :HID])
        nc.vector.tensor_add(ng[:], ng[:], tmpf[:])
        h_prev = gru_state['h']
        h_new = sp.tile([NB, HID], DT.float32, tag="hstep3", name="hnew3")
        for c in range(2):
            cs = slice(c * 128, (c + 1) * 128)
            nc.scalar.activation(ng[:, cs], ng[:, cs], ACTF.Tanh)
            nc.vector.tensor_sub(tmpf[:, cs], h_prev[:, cs], ng[:, cs])
            nc.vector.tensor_mul(tmpf[:, cs],
                                 rz[:, HID + c * 128:HID + (c + 1) * 128],
                                 tmpf[:, cs])
            nc.vector.tensor_add(h_new[:, cs], ng[:, cs], tmpf[:, cs])
            pt = psp.tile([128, NB], DT.float32, tag="ps", name=f"ptf{c}")
            nc.tensor.transpose(pt[:], h_new[:, cs], ident[:])
            nc.vector.tensor_copy(hT[:, c, :], pt[:])
        for hc in range(2):
            for k in range(K):
                nc.tensor.matmul(psk[:, k * B:(k + 1) * B],
                                 hT[:, hc, :], GT[:, hc, k, :],
                                 start=False, stop=(hc == 1 and k == K - 1))
        nc.scalar.activation(y40[:], psk[:], ACTF.Identity)
        nc.sync.dma_start(out_d[:], y40[:])

    nc.compile()
    return nc


def host_prep(inputs):
    """Host-side prep: im2col for conv1, weight layout transforms, bf16 casts."""
    x = np.asarray(inputs['x'], F32)
    xp = np.pad(x, ((0, 0), (0, 0), (0, 0), (2, 2), (2, 2)))
    s = xp.strides
    xs = np.lib.stride_tricks.as_strided(
        xp, shape=(B, T, C, 5, 5, 16, 16),
        strides=(s[0], s[1], s[2], s[3], s[4], 2 * s[3], 2 * s[4]))
    x_col = np.ascontiguousarray(xs).reshape(B, T, 75, NPIX).astype(BF16)

    xcols = []
    for core in range(NCORES):
        xc = x_col[core * NB:(core + 1) * NB]
        arr = np.zeros((NPAIR, 128, 2 * NPIX), BF16)
        for t in range(T):
            for j in range(NB // 2):
                p = t * 4 + j
                arr[p, :75, :NPIX] = xc[2 * j, t]
                arr[p, :75, NPIX:] = xc[2 * j + 1, t]
        xcols.append(arr)

    w = {}
    w1T = np.zeros((128, DIM), BF16)
    w1T[:75] = np.asarray(inputs['enc_w'], F32).reshape(DIM, 75).T.astype(BF16)
    w['w1T'] = w1T
    r1 = np.asarray(inputs['res_w1'], F32).reshape(R, HALF, DIM).transpose(0, 2, 1)
    w['r1T'] = np.ascontiguousarray(
        r1.reshape(R, 4, 128, HALF).transpose(2, 0, 1, 3)).astype(BF16)
    # fp8 DoubleRow weights for resblock 0 (x64, undone by drain scale 2^-6)
    w['r1T8'] = np.ascontiguousarray(
        (r1[0] * 64.0).reshape(4, 128, 2, 128).transpose(1, 0, 2, 3)
        .reshape(128, 2, 2, 2, 128)).astype(E4M3)
    r2 = np.asarray(inputs['res_w2'], F32).transpose(0, 3, 4, 2, 1)
    w['w2T'] = np.ascontiguousarray(
        r2.reshape(R, 9, 2, 128, HALF).transpose(3, 0, 1, 2, 4)).astype(BF16)
    r3 = np.asarray(inputs['res_w3'], F32).reshape(R, DIM, HALF).transpose(0, 2, 1)
    w['r3T'] = np.ascontiguousarray(
        r3.reshape(R, 2, 128, DIM).transpose(2, 0, 1, 3)).astype(BF16)
    w['r3T8'] = np.ascontiguousarray(
        (r3[0] * 64.0).reshape(2, 128, 4, 128).transpose(1, 0, 2, 3)).astype(E4M3)
    w['encb'] = np.ascontiguousarray(
        np.asarray(inputs['enc_b'], F32).reshape(4, 128).T)
    w['b1'] = np.ascontiguousarray(
        np.asarray(inputs['res_b1'], F32).reshape(R, 2, 128).transpose(2, 0, 1))
    w['b2'] = np.ascontiguousarray(
        np.asarray(inputs['res_b2'], F32).reshape(R, 2, 128).transpose(2, 0, 1))
    w['b3'] = np.ascontiguousarray(
        np.asarray(inputs['res_b3'], F32).reshape(R, 4, 128).transpose(2, 0, 1))
    w['gihT'] = np.ascontiguousarray(
        np.asarray(inputs['gru_w_ih'], F32).T.reshape(4, 128, 3 * HID)
        .transpose(1, 0, 2)).astype(BF16)
    w['ghhT'] = np.ascontiguousarray(
        np.asarray(inputs['gru_w_hh'], F32).T.reshape(2, 128, 3 * HID)
        .transpose(1, 0, 2)).astype(BF16)
    bih = np.asarray(inputs['gru_b_ih'], F32)
    bhh = np.asarray(inputs['gru_b_hh'], F32)
    w['gbih'] = np.tile(bih[None, :], (NB, 1))
    w['gbhh'] = np.tile(bhh[None, :], (NB, 1))
    w['gbc'] = np.tile((bih + bhh)[None, :2 * HID], (NB, 1))
    # wkT2[d%128, d//128, k, h] = wk_w[k, d, h]  (for G = ztk . W_k)
    w['wkT2'] = np.ascontiguousarray(
        np.asarray(inputs['wk_w'], F32).reshape(K, 4, 128, HID)
        .transpose(2, 1, 0, 3)).astype(BF16)
    # wkbrep[d%128, d//128, k, i] = wk_b[k, d] (bias via matmul, repl. over i)
    wkb = np.asarray(inputs['wk_b'], F32).reshape(K, 4, 128).transpose(2, 1, 0)
    w['wkbrep'] = np.ascontiguousarray(
        np.repeat(wkb[:, :, :, None], NB, axis=3)).astype(BF16)
    w['ident8'] = np.eye(NB, dtype=F32)
    return xcols, w


_NC_CACHE = {}


def get_nc():
    if 'nc' not in _NC_CACHE:
        _NC_CACHE['nc'] = build_kernel()
    return _NC_CACHE['nc']


def make_in_maps(inputs):
    xcols, w = host_prep(inputs)
    in_maps = []
    for core in range(NCORES):
        m = dict(w)
        m['xcol'] = xcols[core]
        in_maps.append(m)
    return in_maps


def reduce_outputs(results):
    # raw scores s[i, b, k]: anchor i (this core's batch rows), candidate b
    s = np.empty((B, B, K), np.float64)
    for core in range(NCORES):
        o = np.asarray(results[core]['out'], F32)   # [8, K*B]
        s[core * NB:(core + 1) * NB] = (
            o.reshape(NB, K, B).transpose(0, 2, 1))
    # reference: lsm = log_softmax(exp(s), axis=1) over candidates b
    y = np.exp(s)
    m = y.max(axis=1, keepdims=True)
    lse = np.log(np.exp(y - m).sum(axis=1, keepdims=True)) + m
    lsm = y - lse
    idx = np.arange(B)
    loss = np.float32(-lsm[idx, idx, :].mean())
    acc = np.float32((lsm.argmax(axis=1) == idx[:, None]).mean())
    return loss, acc


def _install_ntff_hook():
    """Provide antenv.axon_hooks (missing in this image) so trace=True works."""
    try:
        from antenv.axon_hooks import get_axon_ntff_profile_hook  # noqa: F401
        return
    except ImportError:
        pass
    import ctypes
    import types
    import contextlib

    so_path = "/opt/axon/libaxon_pjrt.so"
    if not os.path.exists(so_path):
        return
    lib = ctypes.CDLL(so_path)
    if not hasattr(lib, "axon_start_nrt_profile"):
        return
    lib.axon_start_nrt_profile.argtypes = [ctypes.POINTER(ctypes.c_int64),
                                           ctypes.c_size_t]
    lib.axon_start_nrt_profile.restype = ctypes.c_int64
    lib.axon_stop_nrt_profile.argtypes = [ctypes.c_char_p]
    lib.axon_stop_nrt_profile.restype = ctypes.c_int64

    @contextlib.contextmanager
    def _hook(output_dir, device_ids):
        import jax
        jax.devices()
        if device_ids:
            ids = (ctypes.c_int64 * len(device_ids))(*device_ids)
            rc = lib.axon_start_nrt_profile(ids, len(device_ids))
        else:
            rc = lib.axon_start_nrt_profile(None, 0)
        if rc != 0:
            raise RuntimeError(f"axon_start_nrt_profile rc={rc}")
        try:
            yield
        finally:
            n = lib.axon_stop_nrt_profile(str(output_dir).encode())
            print(f"ntff profile: {n} file(s) written to {output_dir}")

    mod = types.ModuleType("antenv.axon_hooks")
    mod.get_axon_ntff_profile_hook = lambda: _hook
    mod.set_axon_ntff_profile_hook = lambda h: None
    import antenv
    antenv.axon_hooks = mod
    sys.modules["antenv.axon_hooks"] = mod


def run(inputs, trace=False, **kw):
    if trace:
        _install_ntff_hook()
    nc = get_nc()
    in_maps = make_in_maps(inputs)
    res = run_bass_kernel_spmd(nc, in_maps, core_ids=list(range(NCORES)),
                               trace=trace, **kw)
    return res


def kernel(**inputs):
    res = run(inputs, trace=False)
    return reduce_outputs(res.results)


if __name__ == '__main__':
    import reference as Rf
    inputs = {k: np.asarray(v) for k, v in Rf.setup_inputs().items()}
    loss, acc = kernel(**inputs)
    print('kernel loss/acc:', loss, acc)



# revision 24
# speedup vs baseline: 1.3507x; 1.3507x over previous
"""CPC model (conv encoder + GRU + InfoNCE loss) on 8 TRN2 NeuronCores.

Strategy:
 - Data-parallel over batch: each core owns 8 of 64 sequences (72 images).
 - Conv encoder runs per image-pair as bf16 matmuls (f32 PSUM accum):
     conv1 5x5s2 via host im2col (K=75 padded to 128), resblock 3x3 via
     9-tap shifted matmuls over a zero-padded [18,18] activation.
 - Timestep rows processed in order [4..8, 0..3]: the ztk rows finish early
   so the AllGather overlaps the remaining conv; GRU step t is emitted right
   after row t so it hides under the next row's conv.
 - PSUM->SBUF drains are split across Scalar/Vector/GpSimd engines so the
   in-order scalar queue never head-of-line-blocks the PE; weight DMAs are
   triggered from the (otherwise idle) gpsimd queue for the same reason.
 - Only one activation table (sigmoid/tanh/relu) is used on device; the
   double-exp log-softmax runs on the host in float64 from the raw [8, 320]
   per-core score block (the softmax is tiny; argmax/loss math is exact).
"""
import os
import sys

import numpy as np
import ml_dtypes

for _p in ("/opt/trn_rl_repo", "/root/.axon_site/_ro/trn_rl_repo"):
    if os.path.isdir(_p) and _p not in sys.path:
        sys.path.insert(0, _p)

import concourse.bacc as bacc  # noqa: E402
import concourse.bass as bass  # noqa: E402
import concourse.mybir as mybir  # noqa: E402
import concourse.tile as tile  # noqa: E402
from concourse.bass_utils import run_bass_kernel_spmd  # noqa: E402

F32 = np.float32
BF16 = ml_dtypes.bfloat16
E4M3 = ml_dtypes.float8_e4m3
DT = mybir.dt

B, T, C = 64, 9, 3
DIM, HALF, HID, R, K = 512, 256, 256, 2, 5
TCTX = 4
NCORES = 8
NB = B // NCORES           # 8
NIMG = NB * T              # 72
NPIX = 256                 # 16*16
NPAIR = NIMG // 2          # 36
ROWS = [4, 5, 6, 7, 8, 0, 1, 2, 3]
ALU = mybir.AluOpType
ACTF = mybir.ActivationFunctionType


def build_kernel():
    nc = bacc.Bacc("TRN2", target_bir_lowering=False, debug=False,
                   num_devices=NCORES)

    def din(name, shape, dt):
        return nc.dram_tensor(name, shape, dt, kind="ExternalInput")

    xcol_d = din("xcol", [NPAIR, 128, 512], DT.bfloat16)
    w1T_d = din("w1T", [128, DIM], DT.bfloat16)
    r1T_d = din("r1T", [128, R, 4, HALF], DT.bfloat16)
    # fp8 (e4m3) copies of resblock-0's 1x1 conv weights, x64 scaled,
    # packed for DoubleRow: [part, cpair, kslot, mchunk, 128]
    r1T8_d = din("r1T8", [128, 2, 2, 2, 128], DT.float8e4)
    r3T8_d = din("r3T8", [128, 2, 4, 128], DT.float8e4)
    w2T8_d = din("w2T8", [128, 9, 2, 2, 128], DT.float8e4)
    w2T_d = din("w2T", [128, R, 9, 2, HALF], DT.bfloat16)
    r3T_d = din("r3T", [128, R, 2, DIM], DT.bfloat16)
    encb_d = din("encb", [128, 4], DT.float32)
    b1_d = din("b1", [128, R, 2], DT.float32)
    b2_d = din("b2", [128, R, 2], DT.float32)
    b3_d = din("b3", [128, R, 4], DT.float32)
    gihT_d = din("gihT", [128, 4, 3 * HID], DT.bfloat16)
    ghhT_d = din("ghhT", [128, 2, 3 * HID], DT.bfloat16)
    gbih_d = din("gbih", [NB, 3 * HID], DT.float32)   # host-broadcast over batch
    gbhh_d = din("gbhh", [NB, 3 * HID], DT.float32)
    gbc_d = din("gbc", [NB, 2 * HID], DT.float32)     # (b_ih + b_hh)[: 512]
    wkT2_d = din("wkT2", [128, 4, K, HID], DT.bfloat16)
    wkbrep_d = din("wkbrep", [128, 4, K, NB], DT.bfloat16)
    ident_d = din("ident8", [NB, NB], DT.float32)

    out_d = nc.dram_tensor("out", [NB, K * B], DT.float32, kind="ExternalOutput")

    # collective bounce buffers (internal DRAM, partition-major for cheap DMA)
    zin_b = nc.dram_tensor("zin_b", [128, 4, K * NB], DT.bfloat16)
    zout_b = nc.dram_tensor("zout_b", [NCORES, 128, 4, K, NB], DT.bfloat16,
                            addr_space="Shared")

    from contextlib import ExitStack
    with tile.TileContext(nc) as tc, ExitStack() as stack:
        wp = stack.enter_context(tc.tile_pool(name="weights", bufs=1))
        persist = stack.enter_context(tc.tile_pool(name="persist", bufs=1))
        y1p_pool = stack.enter_context(tc.tile_pool(name="y1p", bufs=4))
        xcp = stack.enter_context(tc.tile_pool(name="xc", bufs=6))
        hp = stack.enter_context(tc.tile_pool(name="h", bufs=8))
        y2p = stack.enter_context(tc.tile_pool(name="y2", bufs=4))
        psp = stack.enter_context(tc.tile_pool(name="psum", bufs=8, space="PSUM"))
        sp = stack.enter_context(tc.tile_pool(name="small", bufs=2))

        # Preload the sigmoid/tanh/relu activation table before any relu so
        # the scalar engine never swaps tables mid-kernel (a swap is 1.28us
        # on the in-order queue and stalls the PE behind PSUM drains).
        junk = persist.tile([NB, 1], DT.float32, tag="junk")
        nc.vector.memset(junk[:], 0.0)
        nc.scalar.activation(junk[:], junk[:], ACTF.Sigmoid)

        # PE warmup: ~30 dummy matmuls on a zero tile keep the PE busy during
        # the initial input/weight DMA wait, so the HAM clock-gate reaches
        # 8/8 before the first real conv matmul (instead of ramping through
        # the first ~3.4us of real work at 1.2 GHz).
        warm = persist.tile([128, 128], DT.bfloat16, tag="warm")
        nc.vector.memset(warm[:], 0.0)
        wps = psp.tile([128, 128], DT.float32, tag="ps", name="warmps")
        for _ in range(30):
            nc.tensor.matmul(wps[:], warm[:], warm[:], start=True, stop=True)

        def wtile(dram, shape, dt, split_dim1=False):
            t = wp.tile(shape, dt, tag=dram.name, name=f"w_{dram.name}")
            if split_dim1:
                for i in range(shape[1]):
                    nc.gpsimd.dma_start(t[:, i], dram[:, i])
            else:
                nc.gpsimd.dma_start(t[:], dram[:])
            return t

        # conv weights first (needed by pair 0); w1T split in four so the
        # first conv1 LDWEIGHTS chunk lands sooner; w2T split per-resblock
        w1T = wp.tile([128, DIM], DT.bfloat16, tag="w1T", name="w_w1T")
        for mc in range(4):
            nc.gpsimd.dma_start(w1T[:, mc * 128:(mc + 1) * 128],
                                w1T_d[:, mc * 128:(mc + 1) * 128])
        encb = wtile(encb_d, [128, 4], DT.float32)
        r1T = wtile(r1T_d, [128, R, 4, HALF], DT.bfloat16)
        r1T8 = wtile(r1T8_d, [128, 2, 2, 2, 128], DT.float8e4)
        r3T8 = wtile(r3T8_d, [128, 2, 4, 128], DT.float8e4)
        w2T8 = wtile(w2T8_d, [128, 9, 2, 2, 128], DT.float8e4)
        b1 = wtile(b1_d, [128, R, 2], DT.float32)
        w2T = wtile(w2T_d, [128, R, 9, 2, HALF], DT.bfloat16, split_dim1=True)
        b2 = wtile(b2_d, [128, R, 2], DT.float32)
        r3T = wtile(r3T_d, [128, R, 2, DIM], DT.bfloat16)
        b3 = wtile(b3_d, [128, R, 4], DT.float32)
        # gru / loss weights (needed only after row t=0)
        gihT = wtile(gihT_d, [128, 4, 3 * HID], DT.bfloat16)
        ghhT = wtile(ghhT_d, [128, 2, 3 * HID], DT.bfloat16)
        gbih = wtile(gbih_d, [NB, 3 * HID], DT.float32)
        gbhh = wtile(gbhh_d, [NB, 3 * HID], DT.float32)
        gbc = wtile(gbc_d, [NB, 2 * HID], DT.float32)
        wkT2 = wtile(wkT2_d, [128, 4, K, HID], DT.bfloat16)
        wkbrep = wtile(wkbrep_d, [128, 4, K, NB], DT.bfloat16)
        ident = wtile(ident_d, [NB, NB], DT.float32)

        zbuf = persist.tile([128, 4, NIMG], DT.bfloat16, tag="zbuf")
        hT = persist.tile([128, 2, NB], DT.bfloat16, tag="hT")
        y40 = persist.tile([NB, K * B], DT.float32, tag="y40")
        # one padded activation tile per (pair parity, resblock): breaks the
        # write-after-read chain between consecutive pairs / resblocks
        # indices (pair_parity*2 + r): r=0 tiles are fp8 (DoubleRow rb_b)
        y1pads = [y1p_pool.tile([128, 2, 2, 18, 18],
                                DT.float8e4 if i % 2 == 0 else DT.bfloat16,
                                tag=("y1pad8" if i % 2 == 0 else "y1pad"),
                                name=f"y1pad{i}")
                  for i in range(4)]
        for ypad in y1pads:
            nc.vector.memset(ypad[:], 0.0)

        # -------- conv encoder: two pairs in lockstep (software pipeline) --
        # Each stage of pair A is followed by the same stage of pair B, so
        # A's PSUM->SBUF drains always have a sibling stage of independent
        # matmuls behind them in the in-order PE queue (no head-of-line
        # stalls on ACT/DVE latency).
        conv1_seq = [0]

        def emit_conv1(p, st):
            xc = xcp.tile([128, 512], DT.bfloat16, tag="xc", name=f"xc{p}")
            seq = conv1_seq[0]
            conv1_seq[0] += 1
            if seq < 4:
                # first-slot pairs gate the kernel head: split each image-pair
                # DMA across two trigger queues (two DMA rings in parallel)
                nc.sync.dma_start(xc[:, :256], xcol_d[p][:, :256])
                nc.scalar.dma_start(xc[:, 256:], xcol_d[p][:, 256:])
            elif seq % 2 == 0:
                nc.sync.dma_start(xc[:], xcol_d[p])
            else:
                nc.scalar.dma_start(xc[:], xcol_d[p])
            h = hp.tile([128, 4, 512], DT.bfloat16, tag="h", name=f"h{p}")
            h8 = hp.tile([128, 4, 512], DT.float8e4, tag="h8", name=f"h8{p}")
            for m in range(4):
                ps = psp.tile([128, 512], DT.float32, tag="ps", name=f"c1ps{p}{m}")
                nc.tensor.matmul(ps[:], w1T[:, m * 128:(m + 1) * 128], xc[:],
                                 start=True, stop=True)
                nc.scalar.activation(h[:, m], ps[:], ACTF.Relu,
                                     bias=encb[:, m:m + 1])
                # fp8 copy feeds resblock-0's DoubleRow rb_a; split ACT/DVE
                # (Pool-engine copies measured ~2us each -- far too slow)
                if m % 2 == 0:
                    nc.scalar.activation(h8[:, m], h[:, m], ACTF.Relu)
                else:
                    nc.vector.tensor_copy(h8[:, m], h[:, m])
            st['h'] = h
            st['h8'] = h8

        def emit_rb_a(p, r, st):
            y1p = y1pads[(p % 2) * 2 + r]
            h = st['h']
            ps_a = [psp.tile([128, 512], DT.float32, tag="ps",
                             name=f"a_ps{p}{r}{m}") for m in range(2)]
            if r == 0:
                # fp8 DoubleRow: 2 k-chunk-pairs x 2 m -> 4 MMs (vs 8 bf16);
                # weights x64 on host, undone by the drain's 2^-6 scale
                h8 = st['h8']
                for m in range(2):
                    for j in range(2):
                        nc.tensor.matmul(
                            ps_a[m][:], r1T8[:, j, :, m, :],
                            h8[:, 2 * j:2 * j + 2, :],
                            start=(j == 0), stop=(j == 1),
                            perf_mode=mybir.MatmulPerfMode.DoubleRow)
            else:
                for m in range(2):
                    for c in range(4):
                        nc.tensor.matmul(ps_a[m][:],
                                         r1T[:, r, c, m * 128:(m + 1) * 128],
                                         h[:, c], start=(c == 0), stop=(c == 3))
            for m in range(2):
                nc.scalar.activation(y1p[:, m, :, 1:17, 1:17],
                                     ps_a[m][:].rearrange("p (i r c) -> p i r c",
                                                          i=2, r=16),
                                     ACTF.Relu, bias=b1[:, r, m:m + 1],
                                     scale=(0.015625 if r == 0 else 1.0))
            st['y1p'] = y1p
            st['y2'] = y2p.tile([128, 2, 512], DT.bfloat16, tag="y2",
                                name=f"y2_{p}{r}")
            if r == 0:
                st['y28'] = y2p.tile([128, 2, 512], DT.float8e4, tag="y28",
                                     name=f"y28_{p}{r}")

        # 3x3 taps ordered center-first: the center tap covers the full 16x16
        # output (start=True clears the bank); edge taps touch only the output
        # rows/cols whose shifted input window stays inside the real image, so
        # the MACs that would multiply pad zeros are skipped entirely
        # (bit-identical: dropping x+0.0 accumulations).
        TAPS = []
        for tap in [4, 0, 1, 2, 3, 5, 6, 7, 8]:
            ky, kx = divmod(tap, 3)
            y0 = 1 if ky == 0 else 0
            x0 = 1 if kx == 0 else 0
            th = 15 if ky != 1 else 16
            tw = 15 if kx != 1 else 16
            TAPS.append((tap, ky, kx, y0, x0, th, tw))

        def emit_rb_b(p, r, m, st):
            y1p, y2 = st['y1p'], st['y2']
            ps = psp.tile([128, 512], DT.float32, tag="ps",
                          name=f"b_ps{p}{r}{m}")
            psv = ps[:].rearrange("p (i r c) -> p i r c", i=2, r=16)
            if r == 0:
                # fp8 DoubleRow conv3x3: the c-pair rides the DoubleRow slot
                # dim; per-image matmuls (ifmap AP is TENSOR3D: 3 free dims)
                for img in range(2):
                    for it, (tap, ky, kx, y0, x0, th, tw) in enumerate(TAPS):
                        rhs = y1p[:, :, img, ky + y0:ky + y0 + th,
                                  kx + x0:kx + x0 + tw]
                        nc.tensor.matmul(
                            psv[:, img, y0:y0 + th, x0:x0 + tw],
                            w2T8[:, tap, :, m, :], rhs,
                            start=(it == 0), stop=(it == 8),
                            perf_mode=mybir.MatmulPerfMode.DoubleRow,
                            skip_group_check=True)
                # drain straight to fp8 (y2 bf16 is unused in resblock 0)
                nc.scalar.activation(st['y28'][:, m], ps[:], ACTF.Relu,
                                     bias=b2[:, r, m:m + 1], scale=0.015625)
                return
            i_mm = 0
            for c in range(2):
                for tap, ky, kx, y0, x0, th, tw in TAPS:
                    rhs = y1p[:, c, :, ky + y0:ky + y0 + th,
                              kx + x0:kx + x0 + tw]
                    nc.tensor.matmul(
                        psv[:, :, y0:y0 + th, x0:x0 + tw],
                        w2T[:, r, tap, c, m * 128:(m + 1) * 128],
                        rhs, start=(i_mm == 0), stop=(i_mm == 17))
                    i_mm += 1
            nc.scalar.activation(y2[:, m], ps[:], ACTF.Relu,
                                 bias=b2[:, r, m:m + 1])

        def emit_rb_c(p, r, st, skip_relu=False):
            h, y2 = st['h'], st['y2']
            hn = hp.tile([128, 4, 512], DT.bfloat16, tag="h", name=f"hn{p}{r}")
            ps3 = [psp.tile([128, 512], DT.float32, tag="ps",
                            name=f"c_ps{p}{r}{m}") for m in range(4)]
            if r == 0:
                # fp8 DoubleRow: one k-pair covers the full 256 contraction
                y28 = st['y28']
                for m in range(4):
                    nc.tensor.matmul(
                        ps3[m][:], r3T8[:, :, m, :], y28[:],
                        start=True, stop=True,
                        perf_mode=mybir.MatmulPerfMode.DoubleRow)
            else:
                # m0/m1 groups complete early so their DVE residual chain
                # overlaps the m2/m3 matmuls
                for m, c in ((0, 0), (1, 0), (0, 1), (1, 1),
                             (2, 0), (3, 0), (2, 1), (3, 1)):
                    nc.tensor.matmul(ps3[m][:],
                                     r3T[:, r, c, m * 128:(m + 1) * 128],
                                     y2[:, c], start=(c == 0), stop=(c == 1))
            for m in range(4):
                if r == 0:
                    # hn = psum * 2^-6 + h (weights were x64 on host)
                    nc.vector.scalar_tensor_tensor(
                        hn[:, m], ps3[m][:], 0.015625, h[:, m],
                        ALU.mult, ALU.add)
                else:
                    nc.vector.tensor_add(hn[:, m], ps3[m][:], h[:, m])
                if skip_relu:
                    continue    # relu fused into the zt accum ACT
                # alternate relu between ACT and DVE: the residual drain
                # is throughput-bound at the rb boundary, so split it
                if m % 2 == 0:
                    nc.scalar.activation(hn[:, m], hn[:, m], ACTF.Relu,
                                         bias=b3[:, r, m:m + 1])
                else:
                    nc.vector.tensor_scalar(hn[:, m], hn[:, m],
                                            b3[:, r, m:m + 1], 0.0,
                                            ALU.add, ALU.max)
            st['h'] = hn

        def emit_zt(p, st):
            t_idx, j_idx = divmod(p, 4)
            col = t_idx * 8 + 2 * j_idx
            h = st['h']
            zt = sp.tile([128, 4, 2], DT.float32, tag="zt", name=f"zt{p}")
            # per-chunk reduce so the last pair's GRU inputs materialize
            # incrementally (chunk c gates only the c-th gi matmul)
            for c in range(4):
                nc.vector.tensor_reduce(
                    zt[:, c], h[:, c].rearrange("p (i x) -> p i x", i=2),
                    mybir.AxisListType.X, ALU.add)
                nc.vector.tensor_scalar_mul(zbuf[:, c, col:col + 2], zt[:, c],
                                            1.0 / NPIX)

        def emit_zt_fused(pA, stA, pB, stB):
            # last-slot variant: relu+bias+spatial-sum fused into one ACT op
            # per (chunk, image) via accum_out, pairs interleaved per chunk,
            # so the final GRU step's zbuf row materializes ~3us sooner
            acc = sp.tile([128, 16], DT.float32, tag="ztacc")
            for c in range(4):
                for i, (p, st) in enumerate(((pA, stA), (pB, stB))):
                    col = (p // 4) * 8 + 2 * (p % 4)
                    h = st['h']
                    for img in range(2):
                        a = acc[:, c * 4 + i * 2 + img:c * 4 + i * 2 + img + 1]
                        nc.scalar.activation(
                            h[:, c, img * NPIX:(img + 1) * NPIX],
                            h[:, c, img * NPIX:(img + 1) * NPIX],
                            ACTF.Relu, bias=b3[:, 1, c:c + 1],
                            accum_out=a)
                        nc.vector.tensor_scalar_mul(
                            zbuf[:, c, col + img:col + img + 1],
                            a, 1.0 / NPIX)

        def emit_slot(pA, pB, stA, stB, nxt=None, hook=None, last=False,
                      nxt_early=False):
            # stA/stB carry this slot's conv1 outputs (prefetched by the
            # previous slot); nxt = (pA', pB', stA', stB') whose conv1 stage
            # is emitted before this slot's final rb-c so the slot-boundary
            # drains always have independent matmuls behind them.
            # nxt_early (first slot only): emit it right after the rb-a
            # stage instead, to cover the cold-start drain stalls.
            for r in range(R):
                emit_rb_a(pA, r, stA)
                emit_rb_a(pB, r, stB)
                if r == 0 and nxt_early and nxt is not None:
                    emit_conv1(nxt[0], nxt[2])
                    emit_conv1(nxt[1], nxt[3])
                emit_rb_b(pA, r, 0, stA)
                emit_rb_b(pB, r, 0, stB)
                if r == 0 and hook is not None:
                    # GRU work for the previous row: emitted ~9us into the
                    # slot so the previous row's zt DVE backlog has drained
                    hook()
                emit_rb_b(pA, r, 1, stA)
                emit_rb_b(pB, r, 1, stB)
                if r == R - 1 and nxt is not None and not nxt_early:
                    emit_conv1(nxt[0], nxt[2])
                    emit_conv1(nxt[1], nxt[3])
                emit_rb_c(pA, r, stA, skip_relu=(last and r == R - 1))
                emit_rb_c(pB, r, stB, skip_relu=(last and r == R - 1))
            if last:
                emit_zt_fused(pA, stA, pB, stB)
            else:
                emit_zt(pA, stA)
                emit_zt(pB, stB)

        # ---------------- GRU step (emitted after row t) ----------------
        gru_state = {'h': None}

        def emit_gru_mms(t):
            gi_rz = psp.tile([NB, 2 * HID], DT.float32, tag="ps", name=f"girz{t}")
            gi_n = psp.tile([NB, HID], DT.float32, tag="ps", name=f"gin{t}")
            for c in range(4):
                nc.tensor.matmul(gi_rz[:], zbuf[:, c, t * 8:(t + 1) * 8],
                                 gihT[:, c, :2 * HID],
                                 start=(c == 0), stop=(c == 3))
            for c in range(4):
                nc.tensor.matmul(gi_n[:], zbuf[:, c, t * 8:(t + 1) * 8],
                                 gihT[:, c, 2 * HID:],
                                 start=(c == 0), stop=(c == 3))
            gh_rz = gh_n = None
            if t > 0:
                gh_rz = psp.tile([NB, 2 * HID], DT.float32, tag="ps",
                                 name=f"ghrz{t}")
                gh_n = psp.tile([NB, HID], DT.float32, tag="ps", name=f"ghn{t}")
                for c in range(2):
                    nc.tensor.matmul(gh_rz[:], hT[:, c, :], ghhT[:, c, :2 * HID],
                                     start=(c == 0), stop=(c == 1))
                for c in range(2):
                    nc.tensor.matmul(gh_n[:], hT[:, c, :], ghhT[:, c, 2 * HID:],
                                     start=(c == 0), stop=(c == 1))
            return gi_rz, gi_n, gh_rz, gh_n

        def emit_gru_chain(t, gi_rz, gi_n, gh_rz, gh_n, split_final=False):
            rz = sp.tile([NB, 2 * HID], DT.float32, tag="rz", name=f"rz{t}")
            ng = sp.tile([NB, HID], DT.float32, tag="ng", name=f"ng{t}")
            tmp = sp.tile([NB, HID], DT.float32, tag="gtmp", name=f"gtmp{t}")
            # r,z = sigmoid(gi_rz + gh_rz + (b_ih + b_hh)[:512])
            nc.vector.tensor_add(rz[:], gi_rz[:], gbc[:])
            if t > 0:
                nc.vector.tensor_add(rz[:], rz[:], gh_rz[:])
            nc.scalar.activation(rz[:], rz[:], ACTF.Sigmoid)
            # n = tanh(gi_n + b_ih_n + r * (gh_n + b_hh_n))
            if t > 0:
                nc.vector.tensor_add(tmp[:], gh_n[:], gbhh[:, 2 * HID:])
            else:
                nc.vector.tensor_copy(tmp[:], gbhh[:, 2 * HID:])
            nc.vector.tensor_mul(tmp[:], tmp[:], rz[:, :HID])
            nc.vector.tensor_add(ng[:], gi_n[:], gbih[:, 2 * HID:])
            nc.vector.tensor_add(ng[:], ng[:], tmp[:])
            nc.scalar.activation(ng[:], ng[:], ACTF.Tanh)
            # h = (1-z)*n + z*h_prev = n + z*(h_prev - n)
            h_new = sp.tile([NB, HID], DT.float32, tag=f"hstep{t}",
                            name=f"hnew{t}")
            if split_final:
                # per-128-chunk so each hT transpose starts as soon as its
                # half of h is ready (shortens the endgame critical path)
                h_prev = gru_state['h']
                for c in range(2):
                    cs = slice(c * 128, (c + 1) * 128)
                    nc.vector.tensor_sub(tmp[:, cs], h_prev[:, cs], ng[:, cs])
                    nc.vector.tensor_mul(tmp[:, cs],
                                         rz[:, HID + c * 128:HID + (c + 1) * 128],
                                         tmp[:, cs])
                    nc.vector.tensor_add(h_new[:, cs], ng[:, cs], tmp[:, cs])
                    pt = psp.tile([128, NB], DT.float32, tag="ps",
                                  name=f"ptf{c}")
                    nc.tensor.transpose(pt[:], h_new[:, cs], ident[:])
                    nc.vector.tensor_copy(hT[:, c, :], pt[:])
            elif t == 0:
                nc.vector.tensor_mul(tmp[:], rz[:, HID:], ng[:])
                nc.vector.tensor_sub(h_new[:], ng[:], tmp[:])
            else:
                nc.vector.tensor_sub(tmp[:], gru_state['h'][:], ng[:])
                nc.vector.tensor_mul(tmp[:], rz[:, HID:], tmp[:])
                nc.vector.tensor_add(h_new[:], ng[:], tmp[:])
            gru_state['h'] = h_new

        def emit_gru_step(t):
            emit_gru_chain(t, *emit_gru_mms(t))

        def emit_transposes(t):
            h_new = gru_state['h']
            for c in range(2):
                pt = psp.tile([128, NB], DT.float32, tag="ps", name=f"pt{t}{c}")
                nc.tensor.transpose(pt[:], h_new[:, c * 128:(c + 1) * 128],
                                    ident[:])
                nc.vector.tensor_copy(hT[:, c, :], pt[:])

        # -------- emit: conv rows with GRU / collective interleaved --------
        # hooks[s] = ops to emit inside slot s of the NEXT row, so GRU work
        # hides under conv instead of stalling the PE queue.
        hooks = {0: [], 1: []}
        # gathered z, contiguous b innermost: ztk[p, dc, k, b]
        ztk = persist.tile([128, 4, K, B], DT.bfloat16, tag="ztk")
        GT = persist.tile([128, 2, K, B], DT.bfloat16, tag="GT")

        def emit_G():
            # G[b,k,h] = sum_d ztk[b,k,d] * wk_w[k,d,h], computed mid-conv
            # once the AllGather lands, so the endgame scores need only the
            # tiny ct x G matmuls after the final GRU step
            for k in range(K):
                for hc in range(2):
                    psg = psp.tile([128, B], DT.float32, tag="ps",
                                   name=f"g{k}{hc}")
                    for dc in range(4):
                        nc.tensor.matmul(
                            psg[:], wkT2[:, dc, k, hc * 128:(hc + 1) * 128],
                            ztk[:, dc, k, :],
                            start=(dc == 0), stop=(dc == 3))
                    nc.vector.tensor_copy(GT[:, hc, k, :], psg[:])

        gru3_prep = {}

        def emit_gru3_prep():
            # step-3 hidden-state matmuls + bias merges, emitted a row early
            # (hT(2) is ready) so only the gi-dependent ops remain at the end
            gh_rz = psp.tile([NB, 2 * HID], DT.float32, tag="ps", name="ghrz3")
            gh_n = psp.tile([NB, HID], DT.float32, tag="ps", name="ghn3")
            for c in range(2):
                nc.tensor.matmul(gh_rz[:], hT[:, c, :], ghhT[:, c, :2 * HID],
                                 start=(c == 0), stop=(c == 1))
            for c in range(2):
                nc.tensor.matmul(gh_n[:], hT[:, c, :], ghhT[:, c, 2 * HID:],
                                 start=(c == 0), stop=(c == 1))
            rzh = sp.tile([NB, 2 * HID], DT.float32, tag="rzh")
            tmp0 = sp.tile([NB, HID], DT.float32, tag="tmp0")
            nc.vector.tensor_add(rzh[:], gh_rz[:], gbc[:])
            nc.vector.tensor_add(tmp0[:], gh_n[:], gbhh[:, 2 * HID:])
            gru3_prep['rzh'] = rzh
            gru3_prep['tmp0'] = tmp0

        def run_hooks(s):
            for fn in hooks[s]:
                fn()
            hooks[s] = []

        # flatten slots so each slot can prefetch the next slot's conv1
        slot_list = []
        for t in ROWS:
            slot_list.append((t * 4 + 0, t * 4 + 1, 0, False))
            slot_list.append((t * 4 + 2, t * 4 + 3, 1, t == 3))
        slot_states = [({}, {}) for _ in slot_list]
        emit_conv1(slot_list[0][0], slot_states[0][0])
        emit_conv1(slot_list[0][1], slot_states[0][1])

        def emit_row_slots(t):
            for s in (0, 1):
                idx = ROWS.index(t) * 2 + s
                pA, pB, hs, last = slot_list[idx]
                nxt = None
                if idx + 1 < len(slot_list):
                    n = slot_list[idx + 1]
                    nxt = (n[0], n[1],
                           slot_states[idx + 1][0], slot_states[idx + 1][1])
                emit_slot(pA, pB, slot_states[idx][0], slot_states[idx][1],
                          nxt=nxt,
                          hook=(lambda s=hs: run_hooks(s)) if hooks[hs] else None,
                          last=last, nxt_early=(idx == 0))

        for t in ROWS:
            emit_row_slots(t)
            if t == 8:
                # ztk rows complete -> AllGather (gpsimd queue, overlaps conv)
                nc.gpsimd.dma_start(zin_b.ap(), zbuf[:, :, TCTX * 8:])
                nc.gpsimd.collective_compute(
                    "AllGather", ALU.bypass,
                    replica_groups=[list(range(NCORES))],
                    ins=[zin_b.ap().opt()], outs=[zout_b.ap().opt()])
                for core in range(NCORES):
                    nc.gpsimd.dma_start(
                        ztk[:, :, :, core * NB:(core + 1) * NB], zout_b[core])
            if t < 3:
                hooks[0].append(lambda t=t: emit_gru_step(t))
                hooks[1].append(lambda t=t: emit_transposes(t))
                if t == 1:
                    # gathered ztk lands mid row 1 -> compute G during row 2
                    hooks[0].append(emit_G)
                if t == 2:
                    hooks[1].append(emit_gru3_prep)

        # ---------------- endgame: GRU step 3 + raw scores ----------------
        # (host does the double-exp log-softmax in float64 from raw scores)
        gi_rz3 = psp.tile([NB, 2 * HID], DT.float32, tag="ps", name="girz3")
        gi_n3 = psp.tile([NB, HID], DT.float32, tag="ps", name="gin3")
        for c in range(4):
            nc.tensor.matmul(gi_rz3[:], zbuf[:, c, 24:32],
                             gihT[:, c, :2 * HID], start=(c == 0), stop=(c == 3))
        for c in range(4):
            nc.tensor.matmul(gi_n3[:], zbuf[:, c, 24:32],
                             gihT[:, c, 2 * HID:], start=(c == 0), stop=(c == 3))
        # single-bank score accumulator: bias + ct matmuls write disjoint
        # 64-col slices of one [8, 320] PSUM tile as one accumulation group
        # (per-element has_written bits make the slices independent)
        psk = psp.tile([NB, K * B], DT.float32, tag="ps", name="sck")
        for k in range(K):
            for dc in range(4):
                nc.tensor.matmul(psk[:, k * B:(k + 1) * B],
                                 wkbrep[:, dc, k, :], ztk[:, dc, k, :],
                                 start=(k == 0 and dc == 0), stop=False)
        # GRU-3 chain with the gh/bias parts pre-merged (rzh, tmp0); r/z and
        # tanh/finale split so each consumer unblocks as early as possible
        rzh, tmp0 = gru3_prep['rzh'], gru3_prep['tmp0']
        rz = sp.tile([NB, 2 * HID], DT.float32, tag="rz", name="rz3")
        ng = sp.tile([NB, HID], DT.float32, tag="ng", name="ng3")
        tmpf = sp.tile([NB, HID], DT.float32, tag="gtmp", name="gtmp3")
        nc.vector.tensor_add(rz[:, :HID], gi_rz3[:, :HID], rzh[:, :HID])
        nc.vector.tensor_add(ng[:], gi_n3[:], gbih[:, 2 * HID:])
        nc.vector.tensor_add(rz[:, HID:], gi_rz3[:, HID:], rzh[:, HID:])
        nc.scalar.activation(rz[:, :HID], rz[:, :HID], ACTF.Sigmoid)
        nc.scalar.activation(rz[:, HID:], rz[:, HID:], ACTF.Sigmoid)
        nc.vector.tensor_mul(tmpf[:], tmp0[:], rz[:, :HID])
        nc.vector.tensor_add(ng[:], ng[:], tmpf[:])
        h_prev = gru_state['h']
        h_new = sp.tile([NB, HID], DT.float32, tag="hstep3", name="hnew3")
        for c in range(2):
            cs = slice(c * 128, (c + 1) * 128)
            nc.scalar.activation(ng[:, cs], ng[:, cs], ACTF.Tanh)
            nc.vector.tensor_sub(tmpf[:, cs], h_prev[:, cs], ng[:, cs])
            nc.vector.tensor_mul(tmpf[:, cs],
                                 rz[:, HID + c * 128:HID + (c + 1) * 128],
                                 tmpf[:, cs])
            nc.vector.tensor_add(h_new[:, cs], ng[:, cs], tmpf[:, cs])
            pt = psp.tile([128, NB], DT.float32, tag="ps", name=f"ptf{c}")
            nc.tensor.transpose(pt[:], h_new[:, cs], ident[:])
            nc.vector.tensor_copy(hT[:, c, :], pt[:])
        for hc in range(2):
            for k in range(K):
                nc.tensor.matmul(psk[:, k * B:(k + 1) * B],
                                 hT[:, hc, :], GT[:, hc, k, :],
                                 start=False, stop=(hc == 1 and k == K - 1))
        nc.scalar.activation(y40[:], psk[:], ACTF.Identity)
        nc.sync.dma_start(out_d[:], y40[:])

    nc.compile()
    return nc


def host_prep(inputs):
    """Host-side prep: im2col for conv1, weight layout transforms, bf16 casts."""
    x = np.asarray(inputs['x'], F32)
    xp = np.pad(x, ((0, 0), (0, 0), (0, 0), (2, 2), (2, 2)))
    s = xp.strides
    xs = np.lib.stride_tricks.as_strided(
        xp, shape=(B, T, C, 5, 5, 16, 16),
        strides=(s[0], s[1], s[2], s[3], s[4], 2 * s[3], 2 * s[4]))
    x_col = np.ascontiguousarray(xs).reshape(B, T, 75, NPIX).astype(BF16)

    xcols = []
    for core in range(NCORES):
        xc = x_col[core * NB:(core + 1) * NB]
        arr = np.zeros((NPAIR, 128, 2 * NPIX), BF16)
        for t in range(T):
            for j in range(NB // 2):
                p = t * 4 + j
                arr[p, :75, :NPIX] = xc[2 * j, t]
                arr[p, :75, NPIX:] = xc[2 * j + 1, t]
        xcols.append(arr)

    w = {}
    w1T = np.zeros((128, DIM), BF16)
    w1T[:75] = np.asarray(inputs['enc_w'], F32).reshape(DIM, 75).T.astype(BF16)
    w['w1T'] = w1T
    r1 = np.asarray(inputs['res_w1'], F32).reshape(R, HALF, DIM).transpose(0, 2, 1)
    w['r1T'] = np.ascontiguousarray(
        r1.reshape(R, 4, 128, HALF).transpose(2, 0, 1, 3)).astype(BF16)
    # fp8 DoubleRow weights for resblock 0 (x64, undone by drain scale 2^-6)
    w['r1T8'] = np.ascontiguousarray(
        (r1[0] * 64.0).reshape(4, 128, 2, 128).transpose(1, 0, 2, 3)
        .reshape(128, 2, 2, 2, 128)).astype(E4M3)
    r2 = np.asarray(inputs['res_w2'], F32).transpose(0, 3, 4, 2, 1)
    w['w2T'] = np.ascontiguousarray(
        r2.reshape(R, 9, 2, 128, HALF).transpose(3, 0, 1, 2, 4)).astype(BF16)
    w['w2T8'] = np.ascontiguousarray(
        (r2[0] * 64.0).reshape(9, 2, 128, 2, 128)
        .transpose(2, 0, 1, 3, 4)).astype(E4M3)
    r3 = np.asarray(inputs['res_w3'], F32).reshape(R, DIM, HALF).transpose(0, 2, 1)
    w['r3T'] = np.ascontiguousarray(
        r3.reshape(R, 2, 128, DIM).transpose(2, 0, 1, 3)).astype(BF16)
    w['r3T8'] = np.ascontiguousarray(
        (r3[0] * 64.0).reshape(2, 128, 4, 128).transpose(1, 0, 2, 3)).astype(E4M3)
    w['encb'] = np.ascontiguousarray(
        np.asarray(inputs['enc_b'], F32).reshape(4, 128).T)
    w['b1'] = np.ascontiguousarray(
        np.asarray(inputs['res_b1'], F32).reshape(R, 2, 128).transpose(2, 0, 1))
    w['b2'] = np.ascontiguousarray(
        np.asarray(inputs['res_b2'], F32).reshape(R, 2, 128).transpose(2, 0, 1))
    w['b3'] = np.ascontiguousarray(
        np.asarray(inputs['res_b3'], F32).reshape(R, 4, 128).transpose(2, 0, 1))
    w['gihT'] = np.ascontiguousarray(
        np.asarray(inputs['gru_w_ih'], F32).T.reshape(4, 128, 3 * HID)
        .transpose(1, 0, 2)).astype(BF16)
    w['ghhT'] = np.ascontiguousarray(
        np.asarray(inputs['gru_w_hh'], F32).T.reshape(2, 128, 3 * HID)
        .transpose(1, 0, 2)).astype(BF16)
    bih = np.asarray(inputs['gru_b_ih'], F32)
    bhh = np.asarray(inputs['gru_b_hh'], F32)
    w['gbih'] = np.tile(bih[None, :], (NB, 1))
    w['gbhh'] = np.tile(bhh[None, :], (NB, 1))
    w['gbc'] = np.tile((bih + bhh)[None, :2 * HID], (NB, 1))
    # wkT2[d%128, d//128, k, h] = wk_w[k, d, h]  (for G = ztk . W_k)
    w['wkT2'] = np.ascontiguousarray(
        np.asarray(inputs['wk_w'], F32).reshape(K, 4, 128, HID)
        .transpose(2, 1, 0, 3)).astype(BF16)
    # wkbrep[d%128, d//128, k, i] = wk_b[k, d] (bias via matmul, repl. over i)
    wkb = np.asarray(inputs['wk_b'], F32).reshape(K, 4, 128).transpose(2, 1, 0)
    w['wkbrep'] = np.ascontiguousarray(
        np.repeat(wkb[:, :, :, None], NB, axis=3)).astype(BF16)
    w['ident8'] = np.eye(NB, dtype=F32)
    return xcols, w


_NC_CACHE = {}


def get_nc():
    if 'nc' not in _NC_CACHE:
        _NC_CACHE['nc'] = build_kernel()
    return _NC_CACHE['nc']


def make_in_maps(inputs):
    xcols, w = host_prep(inputs)
    in_maps = []
    for core in range(NCORES):
        m = dict(w)
        m['xcol'] = xcols[core]
        in_maps.append(m)
    return in_maps


def reduce_outputs(results):
    # raw scores s[i, b, k]: anchor i (this core's batch rows), candidate b
    s = np.empty((B, B, K), np.float64)
    for core in range(NCORES):
        o = np.asarray(results[core]['out'], F32)   # [8, K*B]
        s[core * NB:(core + 1) * NB] = (
            o.reshape(NB, K, B).transpose(0, 2, 1))
    # reference: lsm = log_softmax(exp(s), axis=1) over candidates b
    y = np.exp(s)
    m = y.max(axis=1, keepdims=True)
    lse = np.log(np.exp(y - m).sum(axis=1, keepdims=True)) + m
    lsm = y - lse
    idx = np.arange(B)
    loss = np.float32(-lsm[idx, idx, :].mean())
    acc = np.float32((lsm.argmax(axis=1) == idx[:, None]).mean())
    return loss, acc


def _install_ntff_hook():
    """Provide antenv.axon_hooks (missing in this image) so trace=True works."""
    try:
        from antenv.axon_hooks import get_axon_ntff_profile_hook  # noqa: F401
        return
    except ImportError:
        pass
    import ctypes
    import types
    import contextlib

    so_path = "/opt/axon/libaxon_pjrt.so"
    if not os.path.exists(so_path):
        return
    lib = ctypes.CDLL(so_path)
    if not hasattr(lib, "axon_start_nrt_profile"):
        return
    lib.axon_start_nrt_profile.argtypes = [ctypes.POINTER(ctypes.c_int64),
                                           ctypes.c_size_t]
    lib.axon_start_nrt_profile.restype = ctypes.c_int64
    lib.axon_stop_nrt_profile.argtypes = [ctypes.c_char_p]
    lib.axon_stop_nrt_profile.restype = ctypes.c_int64

    @contextlib.contextmanager
    def _hook(output_dir, device_ids):
        import jax
        jax.devices()
        if device_ids:
            ids = (ctypes.c_int64 * len(device_ids))(*device_ids)
            rc = lib.axon_start_nrt_profile(ids, len(device_ids))
        else:
            rc = lib.axon_start_nrt_profile(None, 0)
        if rc != 0:
            raise RuntimeError(f"axon_start_nrt_profile rc={rc}")
        try:
            yield
        finally:
            n = lib.axon_stop_nrt_profile(str(output_dir).encode())
            print(f"ntff profile: {n} file(s) written to {output_dir}")

    mod = types.ModuleType("antenv.axon_hooks")
    mod.get_axon_ntff_profile_hook = lambda: _hook
    mod.set_axon_ntff_profile_hook = lambda h: None
    import antenv
    antenv.axon_hooks = mod
    sys.modules["antenv.axon_hooks"] = mod


def run(inputs, trace=False, **kw):
    if trace:
        _install_ntff_hook()
    nc = get_nc()
    in_maps = make_in_maps(inputs)
    res = run_bass_kernel_spmd(nc, in_maps, core_ids=list(range(NCORES)),
                               trace=trace, **kw)
    return res


def kernel(**inputs):
    res = run(inputs, trace=False)
    return reduce_outputs(res.results)


if __name__ == '__main__':
    import reference as Rf
    inputs = {k: np.asarray(v) for k, v in Rf.setup_inputs().items()}
    loss, acc = kernel(**inputs)
    print('kernel loss/acc:', loss, acc)



# revision 26
# speedup vs baseline: 1.4790x; 1.0949x over previous
"""CPC model (conv encoder + GRU + InfoNCE loss) on 8 TRN2 NeuronCores.

Strategy:
 - Data-parallel over batch: each core owns 8 of 64 sequences (72 images).
 - Conv encoder runs per image-pair as bf16 matmuls (f32 PSUM accum):
     conv1 5x5s2 via host im2col (K=75 padded to 128), resblock 3x3 via
     9-tap shifted matmuls over a zero-padded [18,18] activation.
 - Timestep rows processed in order [4..8, 0..3]: the ztk rows finish early
   so the AllGather overlaps the remaining conv; GRU step t is emitted right
   after row t so it hides under the next row's conv.
 - PSUM->SBUF drains are split across Scalar/Vector/GpSimd engines so the
   in-order scalar queue never head-of-line-blocks the PE; weight DMAs are
   triggered from the (otherwise idle) gpsimd queue for the same reason.
 - Only one activation table (sigmoid/tanh/relu) is used on device; the
   double-exp log-softmax runs on the host in float64 from the raw [8, 320]
   per-core score block (the softmax is tiny; argmax/loss math is exact).
"""
import os
import sys

import numpy as np
import ml_dtypes

for _p in ("/opt/trn_rl_repo", "/root/.axon_site/_ro/trn_rl_repo"):
    if os.path.isdir(_p) and _p not in sys.path:
        sys.path.insert(0, _p)

import concourse.bacc as bacc  # noqa: E402
import concourse.bass as bass  # noqa: E402
import concourse.mybir as mybir  # noqa: E402
import concourse.tile as tile  # noqa: E402
from concourse.bass_utils import run_bass_kernel_spmd  # noqa: E402

F32 = np.float32
BF16 = ml_dtypes.bfloat16
E4M3 = ml_dtypes.float8_e4m3
DT = mybir.dt

B, T, C = 64, 9, 3
DIM, HALF, HID, R, K = 512, 256, 256, 2, 5
TCTX = 4
NCORES = 8
NB = B // NCORES           # 8
NIMG = NB * T              # 72
NPIX = 256                 # 16*16
NPAIR = NIMG // 2          # 36
ROWS = [4, 5, 6, 7, 8, 0, 1, 2, 3]
ALU = mybir.AluOpType
ACTF = mybir.ActivationFunctionType


def build_kernel():
    nc = bacc.Bacc("TRN2", target_bir_lowering=False, debug=False,
                   num_devices=NCORES)

    def din(name, shape, dt):
        return nc.dram_tensor(name, shape, dt, kind="ExternalInput")

    xcol_d = din("xcol", [NPAIR, 128, 512], DT.bfloat16)
    w1T_d = din("w1T", [128, DIM], DT.bfloat16)
    r1T_d = din("r1T", [128, R, 4, HALF], DT.bfloat16)
    # fp8 (e4m3) copies of resblock-0's 1x1 conv weights, x64 scaled,
    # packed for DoubleRow: [part, cpair, kslot, mchunk, 128]
    r1T8_d = din("r1T8", [128, 2, 2, 2, 128], DT.float8e4)
    r3T8_d = din("r3T8", [128, 2, 4, 128], DT.float8e4)
    w2T8_d = din("w2T8", [128, 9, 2, 2, 128], DT.float8e4)
    w2T_d = din("w2T", [128, R, 9, 2, HALF], DT.bfloat16)
    r3T_d = din("r3T", [128, R, 2, DIM], DT.bfloat16)
    encb_d = din("encb", [128, 4], DT.float32)
    b1_d = din("b1", [128, R, 2], DT.float32)
    b2_d = din("b2", [128, R, 2], DT.float32)
    b3_d = din("b3", [128, R, 4], DT.float32)
    gihT_d = din("gihT", [128, 4, 3 * HID], DT.bfloat16)
    ghhT_d = din("ghhT", [128, 2, 3 * HID], DT.bfloat16)
    gbih_d = din("gbih", [NB, 3 * HID], DT.float32)   # host-broadcast over batch
    gbhh_d = din("gbhh", [NB, 3 * HID], DT.float32)
    gbc_d = din("gbc", [NB, 2 * HID], DT.float32)     # (b_ih + b_hh)[: 512]
    wkT2_d = din("wkT2", [128, 4, K, HID], DT.bfloat16)
    wkbrep_d = din("wkbrep", [128, 4, K, NB], DT.bfloat16)
    ident_d = din("ident8", [NB, NB], DT.float32)

    out_d = nc.dram_tensor("out", [NB, K * B], DT.float32, kind="ExternalOutput")

    # collective bounce buffers (internal DRAM, partition-major for cheap DMA)
    zin_b = nc.dram_tensor("zin_b", [128, 4, K * NB], DT.bfloat16)
    zout_b = nc.dram_tensor("zout_b", [NCORES, 128, 4, K, NB], DT.bfloat16,
                            addr_space="Shared")

    from contextlib import ExitStack
    with tile.TileContext(nc) as tc, ExitStack() as stack:
        wp = stack.enter_context(tc.tile_pool(name="weights", bufs=1))
        persist = stack.enter_context(tc.tile_pool(name="persist", bufs=1))
        y1p_pool = stack.enter_context(tc.tile_pool(name="y1p", bufs=4))
        xcp = stack.enter_context(tc.tile_pool(name="xc", bufs=6))
        hp = stack.enter_context(tc.tile_pool(name="h", bufs=8))
        y2p = stack.enter_context(tc.tile_pool(name="y2", bufs=4))
        psp = stack.enter_context(tc.tile_pool(name="psum", bufs=8, space="PSUM"))
        sp = stack.enter_context(tc.tile_pool(name="small", bufs=2))

        # Preload the sigmoid/tanh/relu activation table before any relu so
        # the scalar engine never swaps tables mid-kernel (a swap is 1.28us
        # on the in-order queue and stalls the PE behind PSUM drains).
        junk = persist.tile([NB, 1], DT.float32, tag="junk")
        nc.vector.memset(junk[:], 0.0)
        nc.scalar.activation(junk[:], junk[:], ACTF.Sigmoid)

        # PE warmup: ~30 dummy matmuls on a zero tile keep the PE busy during
        # the initial input/weight DMA wait, so the HAM clock-gate reaches
        # 8/8 before the first real conv matmul (instead of ramping through
        # the first ~3.4us of real work at 1.2 GHz).
        warm = persist.tile([128, 128], DT.bfloat16, tag="warm")
        nc.vector.memset(warm[:], 0.0)
        wps = psp.tile([128, 128], DT.float32, tag="ps", name="warmps")
        for _ in range(30):
            nc.tensor.matmul(wps[:], warm[:], warm[:], start=True, stop=True)

        def wtile(dram, shape, dt, split_dim1=False):
            t = wp.tile(shape, dt, tag=dram.name, name=f"w_{dram.name}")
            if split_dim1:
                for i in range(shape[1]):
                    nc.gpsimd.dma_start(t[:, i], dram[:, i])
            else:
                nc.gpsimd.dma_start(t[:], dram[:])
            return t

        # conv weights first (needed by pair 0); w1T split in four so the
        # first conv1 LDWEIGHTS chunk lands sooner; w2T split per-resblock
        w1T = wp.tile([128, DIM], DT.bfloat16, tag="w1T", name="w_w1T")
        for mc in range(4):
            nc.gpsimd.dma_start(w1T[:, mc * 128:(mc + 1) * 128],
                                w1T_d[:, mc * 128:(mc + 1) * 128])
        encb = wtile(encb_d, [128, 4], DT.float32)
        r1T = wtile(r1T_d, [128, R, 4, HALF], DT.bfloat16)
        r1T8 = wtile(r1T8_d, [128, 2, 2, 2, 128], DT.float8e4)
        r3T8 = wtile(r3T8_d, [128, 2, 4, 128], DT.float8e4)
        w2T8 = wtile(w2T8_d, [128, 9, 2, 2, 128], DT.float8e4)
        b1 = wtile(b1_d, [128, R, 2], DT.float32)
        w2T = wtile(w2T_d, [128, R, 9, 2, HALF], DT.bfloat16, split_dim1=True)
        b2 = wtile(b2_d, [128, R, 2], DT.float32)
        r3T = wtile(r3T_d, [128, R, 2, DIM], DT.bfloat16)
        b3 = wtile(b3_d, [128, R, 4], DT.float32)
        # gru / loss weights (needed only after row t=0)
        gihT = wtile(gihT_d, [128, 4, 3 * HID], DT.bfloat16)
        ghhT = wtile(ghhT_d, [128, 2, 3 * HID], DT.bfloat16)
        gbih = wtile(gbih_d, [NB, 3 * HID], DT.float32)
        gbhh = wtile(gbhh_d, [NB, 3 * HID], DT.float32)
        gbc = wtile(gbc_d, [NB, 2 * HID], DT.float32)
        wkT2 = wtile(wkT2_d, [128, 4, K, HID], DT.bfloat16)
        wkbrep = wtile(wkbrep_d, [128, 4, K, NB], DT.bfloat16)
        ident = wtile(ident_d, [NB, NB], DT.float32)

        zbuf = persist.tile([128, 4, NIMG], DT.bfloat16, tag="zbuf")
        hT = persist.tile([128, 2, NB], DT.bfloat16, tag="hT")
        y40 = persist.tile([NB, K * B], DT.float32, tag="y40")
        # one padded activation tile per (pair parity, resblock): breaks the
        # write-after-read chain between consecutive pairs / resblocks
        # indices (pair_parity*2 + r): r=0 tiles are fp8 (DoubleRow rb_b)
        y1pads = [y1p_pool.tile([128, 2, 2, 18, 18],
                                DT.float8e4 if i % 2 == 0 else DT.bfloat16,
                                tag=("y1pad8" if i % 2 == 0 else "y1pad"),
                                name=f"y1pad{i}")
                  for i in range(4)]
        for ypad in y1pads:
            nc.vector.memset(ypad[:], 0.0)

        # -------- conv encoder: two pairs in lockstep (software pipeline) --
        # Each stage of pair A is followed by the same stage of pair B, so
        # A's PSUM->SBUF drains always have a sibling stage of independent
        # matmuls behind them in the in-order PE queue (no head-of-line
        # stalls on ACT/DVE latency).
        conv1_seq = [0]

        def emit_conv1(p, st):
            xc = xcp.tile([128, 512], DT.bfloat16, tag="xc", name=f"xc{p}")
            seq = conv1_seq[0]
            conv1_seq[0] += 1
            if seq < 4:
                # first-slot pairs gate the kernel head: split each image-pair
                # DMA across two trigger queues (two DMA rings in parallel)
                nc.sync.dma_start(xc[:, :256], xcol_d[p][:, :256])
                nc.scalar.dma_start(xc[:, 256:], xcol_d[p][:, 256:])
            elif seq % 2 == 0:
                nc.sync.dma_start(xc[:], xcol_d[p])
            else:
                nc.scalar.dma_start(xc[:], xcol_d[p])
            h = hp.tile([128, 4, 512], DT.bfloat16, tag="h", name=f"h{p}")
            h8 = hp.tile([128, 4, 512], DT.float8e4, tag="h8", name=f"h8{p}")
            for m in range(4):
                ps = psp.tile([128, 512], DT.float32, tag="ps", name=f"c1ps{p}{m}")
                nc.tensor.matmul(ps[:], w1T[:, m * 128:(m + 1) * 128], xc[:],
                                 start=True, stop=True)
                nc.scalar.activation(h[:, m], ps[:], ACTF.Relu,
                                     bias=encb[:, m:m + 1])
                # fp8 copy feeds resblock-0's DoubleRow rb_a; on DVE (the
                # ACT queue gates the r0 drains; Pool copies are ~2us each)
                nc.vector.tensor_copy(h8[:, m], h[:, m])
            st['h'] = h
            st['h8'] = h8

        def emit_rb_a(p, r, st):
            y1p = y1pads[(p % 2) * 2 + r]
            h = st['h']
            ps_a = [psp.tile([128, 512], DT.float32, tag="ps",
                             name=f"a_ps{p}{r}{m}") for m in range(2)]
            if r == 0:
                # fp8 DoubleRow: 2 k-chunk-pairs x 2 m -> 4 MMs (vs 8 bf16);
                # weights x64 on host, undone by the drain's 2^-6 scale
                h8 = st['h8']
                for m in range(2):
                    for j in range(2):
                        nc.tensor.matmul(
                            ps_a[m][:], r1T8[:, j, :, m, :],
                            h8[:, 2 * j:2 * j + 2, :],
                            start=(j == 0), stop=(j == 1),
                            perf_mode=mybir.MatmulPerfMode.DoubleRow)
            else:
                for m in range(2):
                    for c in range(4):
                        nc.tensor.matmul(ps_a[m][:],
                                         r1T[:, r, c, m * 128:(m + 1) * 128],
                                         h[:, c], start=(c == 0), stop=(c == 3))
            for m in range(2):
                nc.scalar.activation(y1p[:, m, :, 1:17, 1:17],
                                     ps_a[m][:].rearrange("p (i r c) -> p i r c",
                                                          i=2, r=16),
                                     ACTF.Relu, bias=b1[:, r, m:m + 1],
                                     scale=(0.015625 if r == 0 else 1.0))
            st['y1p'] = y1p
            st['y2'] = y2p.tile([128, 2, 512], DT.bfloat16, tag="y2",
                                name=f"y2_{p}{r}")
            if r == 0:
                st['y28'] = y2p.tile([128, 2, 512], DT.float8e4, tag="y28",
                                     name=f"y28_{p}{r}")

        # 3x3 taps ordered center-first: the center tap covers the full 16x16
        # output (start=True clears the bank); edge taps touch only the output
        # rows/cols whose shifted input window stays inside the real image, so
        # the MACs that would multiply pad zeros are skipped entirely
        # (bit-identical: dropping x+0.0 accumulations).
        TAPS = []
        for tap in [4, 0, 1, 2, 3, 5, 6, 7, 8]:
            ky, kx = divmod(tap, 3)
            y0 = 1 if ky == 0 else 0
            x0 = 1 if kx == 0 else 0
            th = 15 if ky != 1 else 16
            tw = 15 if kx != 1 else 16
            TAPS.append((tap, ky, kx, y0, x0, th, tw))

        def emit_rb_b(p, r, m, st):
            y1p, y2 = st['y1p'], st['y2']
            ps = psp.tile([128, 512], DT.float32, tag="ps",
                          name=f"b_ps{p}{r}{m}")
            psv = ps[:].rearrange("p (i r c) -> p i r c", i=2, r=16)
            if r == 0:
                # fp8 DoubleRow conv3x3: the c-pair rides the DoubleRow slot
                # dim; per-image matmuls (ifmap AP is TENSOR3D: 3 free dims)
                for img in range(2):
                    for it, (tap, ky, kx, y0, x0, th, tw) in enumerate(TAPS):
                        rhs = y1p[:, :, img, ky + y0:ky + y0 + th,
                                  kx + x0:kx + x0 + tw]
                        nc.tensor.matmul(
                            psv[:, img, y0:y0 + th, x0:x0 + tw],
                            w2T8[:, tap, :, m, :], rhs,
                            start=(it == 0), stop=(it == 8),
                            perf_mode=mybir.MatmulPerfMode.DoubleRow,
                            skip_group_check=True)
                # drain straight to fp8 (y2 bf16 is unused in resblock 0)
                nc.scalar.activation(st['y28'][:, m], ps[:], ACTF.Relu,
                                     bias=b2[:, r, m:m + 1], scale=0.015625)
                return
            i_mm = 0
            for c in range(2):
                for tap, ky, kx, y0, x0, th, tw in TAPS:
                    rhs = y1p[:, c, :, ky + y0:ky + y0 + th,
                              kx + x0:kx + x0 + tw]
                    nc.tensor.matmul(
                        psv[:, :, y0:y0 + th, x0:x0 + tw],
                        w2T[:, r, tap, c, m * 128:(m + 1) * 128],
                        rhs, start=(i_mm == 0), stop=(i_mm == 17))
                    i_mm += 1
            nc.scalar.activation(y2[:, m], ps[:], ACTF.Relu,
                                 bias=b2[:, r, m:m + 1])

        def emit_rb_c(p, r, st, skip_relu=False):
            h, y2 = st['h'], st['y2']
            hn = hp.tile([128, 4, 512], DT.bfloat16, tag="h", name=f"hn{p}{r}")
            ps3 = [psp.tile([128, 512], DT.float32, tag="ps",
                            name=f"c_ps{p}{r}{m}") for m in range(4)]
            if r == 0:
                # fp8 DoubleRow: one k-pair covers the full 256 contraction
                y28 = st['y28']
                for m in range(4):
                    nc.tensor.matmul(
                        ps3[m][:], r3T8[:, :, m, :], y28[:],
                        start=True, stop=True,
                        perf_mode=mybir.MatmulPerfMode.DoubleRow)
            else:
                # m0/m1 groups complete early so their DVE residual chain
                # overlaps the m2/m3 matmuls
                for m, c in ((0, 0), (1, 0), (0, 1), (1, 1),
                             (2, 0), (3, 0), (2, 1), (3, 1)):
                    nc.tensor.matmul(ps3[m][:],
                                     r3T[:, r, c, m * 128:(m + 1) * 128],
                                     y2[:, c], start=(c == 0), stop=(c == 1))
            for m in range(4):
                if r == 0:
                    # hn = psum * 2^-6 + h (weights were x64 on host)
                    nc.vector.scalar_tensor_tensor(
                        hn[:, m], ps3[m][:], 0.015625, h[:, m],
                        ALU.mult, ALU.add)
                else:
                    nc.vector.tensor_add(hn[:, m], ps3[m][:], h[:, m])
                if skip_relu:
                    continue    # relu fused into the zt accum ACT
                # alternate relu between ACT and DVE: the residual drain
                # is throughput-bound at the rb boundary, so split it
                if m % 2 == 0:
                    nc.scalar.activation(hn[:, m], hn[:, m], ACTF.Relu,
                                         bias=b3[:, r, m:m + 1])
                else:
                    nc.vector.tensor_scalar(hn[:, m], hn[:, m],
                                            b3[:, r, m:m + 1], 0.0,
                                            ALU.add, ALU.max)
            st['h'] = hn

        def emit_zt(p, st):
            t_idx, j_idx = divmod(p, 4)
            col = t_idx * 8 + 2 * j_idx
            h = st['h']
            zt = sp.tile([128, 4, 2], DT.float32, tag="zt", name=f"zt{p}")
            # per-chunk reduce so the last pair's GRU inputs materialize
            # incrementally (chunk c gates only the c-th gi matmul)
            for c in range(4):
                nc.vector.tensor_reduce(
                    zt[:, c], h[:, c].rearrange("p (i x) -> p i x", i=2),
                    mybir.AxisListType.X, ALU.add)
                nc.vector.tensor_scalar_mul(zbuf[:, c, col:col + 2], zt[:, c],
                                            1.0 / NPIX)

        def emit_zt_fused(pA, stA, pB, stB):
            # last-slot variant: relu+bias+spatial-sum fused into one ACT op
            # per (chunk, image) via accum_out, pairs interleaved per chunk,
            # so the final GRU step's zbuf row materializes ~3us sooner
            acc = sp.tile([128, 16], DT.float32, tag="ztacc")
            for c in range(4):
                for i, (p, st) in enumerate(((pA, stA), (pB, stB))):
                    col = (p // 4) * 8 + 2 * (p % 4)
                    h = st['h']
                    for img in range(2):
                        a = acc[:, c * 4 + i * 2 + img:c * 4 + i * 2 + img + 1]
                        nc.scalar.activation(
                            h[:, c, img * NPIX:(img + 1) * NPIX],
                            h[:, c, img * NPIX:(img + 1) * NPIX],
                            ACTF.Relu, bias=b3[:, 1, c:c + 1],
                            accum_out=a)
                        nc.vector.tensor_scalar_mul(
                            zbuf[:, c, col + img:col + img + 1],
                            a, 1.0 / NPIX)

        def emit_slot(pA, pB, stA, stB, nxt=None, hook=None, last=False,
                      nxt_early=False):
            # stA/stB carry this slot's conv1 outputs (prefetched by the
            # previous slot); nxt = (pA', pB', stA', stB') whose conv1 stage
            # is emitted before this slot's final rb-c so the slot-boundary
            # drains always have independent matmuls behind them.
            # nxt_early (first slot only): emit it right after the rb-a
            # stage instead, to cover the cold-start drain stalls.
            for r in range(R):
                emit_rb_a(pA, r, stA)
                emit_rb_a(pB, r, stB)
                if r == 0 and nxt_early and nxt is not None:
                    emit_conv1(nxt[0], nxt[2])
                    emit_conv1(nxt[1], nxt[3])
                emit_rb_b(pA, r, 0, stA)
                emit_rb_b(pB, r, 0, stB)
                if r == 1 and hook is not None:
                    # GRU work for the previous row: emitted mid-r1 (bf16
                    # conv) so its ACT ops never queue ahead of the fp8 r0
                    # drains that the DoubleRow matmuls wait on
                    hook()
                emit_rb_b(pA, r, 1, stA)
                emit_rb_b(pB, r, 1, stB)
                if r == R - 1 and nxt is not None and not nxt_early:
                    emit_conv1(nxt[0], nxt[2])
                    emit_conv1(nxt[1], nxt[3])
                emit_rb_c(pA, r, stA, skip_relu=(last and r == R - 1))
                emit_rb_c(pB, r, stB, skip_relu=(last and r == R - 1))
            if last:
                emit_zt_fused(pA, stA, pB, stB)
            else:
                emit_zt(pA, stA)
                emit_zt(pB, stB)

        # ---------------- GRU step (emitted after row t) ----------------
        gru_state = {'h': None}

        def emit_gru_mms(t):
            gi_rz = psp.tile([NB, 2 * HID], DT.float32, tag="ps", name=f"girz{t}")
            gi_n = psp.tile([NB, HID], DT.float32, tag="ps", name=f"gin{t}")
            for c in range(4):
                nc.tensor.matmul(gi_rz[:], zbuf[:, c, t * 8:(t + 1) * 8],
                                 gihT[:, c, :2 * HID],
                                 start=(c == 0), stop=(c == 3))
            for c in range(4):
                nc.tensor.matmul(gi_n[:], zbuf[:, c, t * 8:(t + 1) * 8],
                                 gihT[:, c, 2 * HID:],
                                 start=(c == 0), stop=(c == 3))
            gh_rz = gh_n = None
            if t > 0:
                gh_rz = psp.tile([NB, 2 * HID], DT.float32, tag="ps",
                                 name=f"ghrz{t}")
                gh_n = psp.tile([NB, HID], DT.float32, tag="ps", name=f"ghn{t}")
                for c in range(2):
                    nc.tensor.matmul(gh_rz[:], hT[:, c, :], ghhT[:, c, :2 * HID],
                                     start=(c == 0), stop=(c == 1))
                for c in range(2):
                    nc.tensor.matmul(gh_n[:], hT[:, c, :], ghhT[:, c, 2 * HID:],
                                     start=(c == 0), stop=(c == 1))
            return gi_rz, gi_n, gh_rz, gh_n

        def emit_gru_chain(t, gi_rz, gi_n, gh_rz, gh_n, split_final=False):
            rz = sp.tile([NB, 2 * HID], DT.float32, tag="rz", name=f"rz{t}")
            ng = sp.tile([NB, HID], DT.float32, tag="ng", name=f"ng{t}")
            tmp = sp.tile([NB, HID], DT.float32, tag="gtmp", name=f"gtmp{t}")
            # r,z = sigmoid(gi_rz + gh_rz + (b_ih + b_hh)[:512])
            nc.vector.tensor_add(rz[:], gi_rz[:], gbc[:])
            if t > 0:
                nc.vector.tensor_add(rz[:], rz[:], gh_rz[:])
            nc.scalar.activation(rz[:], rz[:], ACTF.Sigmoid)
            # n = tanh(gi_n + b_ih_n + r * (gh_n + b_hh_n))
            if t > 0:
                nc.vector.tensor_add(tmp[:], gh_n[:], gbhh[:, 2 * HID:])
            else:
                nc.vector.tensor_copy(tmp[:], gbhh[:, 2 * HID:])
            nc.vector.tensor_mul(tmp[:], tmp[:], rz[:, :HID])
            nc.vector.tensor_add(ng[:], gi_n[:], gbih[:, 2 * HID:])
            nc.vector.tensor_add(ng[:], ng[:], tmp[:])
            nc.scalar.activation(ng[:], ng[:], ACTF.Tanh)
            # h = (1-z)*n + z*h_prev = n + z*(h_prev - n)
            h_new = sp.tile([NB, HID], DT.float32, tag=f"hstep{t}",
                            name=f"hnew{t}")
            if split_final:
                # per-128-chunk so each hT transpose starts as soon as its
                # half of h is ready (shortens the endgame critical path)
                h_prev = gru_state['h']
                for c in range(2):
                    cs = slice(c * 128, (c + 1) * 128)
                    nc.vector.tensor_sub(tmp[:, cs], h_prev[:, cs], ng[:, cs])
                    nc.vector.tensor_mul(tmp[:, cs],
                                         rz[:, HID + c * 128:HID + (c + 1) * 128],
                                         tmp[:, cs])
                    nc.vector.tensor_add(h_new[:, cs], ng[:, cs], tmp[:, cs])
                    pt = psp.tile([128, NB], DT.float32, tag="ps",
                                  name=f"ptf{c}")
                    nc.tensor.transpose(pt[:], h_new[:, cs], ident[:])
                    nc.vector.tensor_copy(hT[:, c, :], pt[:])
            elif t == 0:
                nc.vector.tensor_mul(tmp[:], rz[:, HID:], ng[:])
                nc.vector.tensor_sub(h_new[:], ng[:], tmp[:])
            else:
                nc.vector.tensor_sub(tmp[:], gru_state['h'][:], ng[:])
                nc.vector.tensor_mul(tmp[:], rz[:, HID:], tmp[:])
                nc.vector.tensor_add(h_new[:], ng[:], tmp[:])
            gru_state['h'] = h_new

        def emit_gru_step(t):
            emit_gru_chain(t, *emit_gru_mms(t))

        def emit_transposes(t):
            h_new = gru_state['h']
            for c in range(2):
                pt = psp.tile([128, NB], DT.float32, tag="ps", name=f"pt{t}{c}")
                nc.tensor.transpose(pt[:], h_new[:, c * 128:(c + 1) * 128],
                                    ident[:])
                nc.vector.tensor_copy(hT[:, c, :], pt[:])

        # -------- emit: conv rows with GRU / collective interleaved --------
        # hooks[s] = ops to emit inside slot s of the NEXT row, so GRU work
        # hides under conv instead of stalling the PE queue.
        hooks = {0: [], 1: []}
        # gathered z, contiguous b innermost: ztk[p, dc, k, b]
        ztk = persist.tile([128, 4, K, B], DT.bfloat16, tag="ztk")
        GT = persist.tile([128, 2, K, B], DT.bfloat16, tag="GT")

        def emit_G():
            # G[b,k,h] = sum_d ztk[b,k,d] * wk_w[k,d,h], computed mid-conv
            # once the AllGather lands, so the endgame scores need only the
            # tiny ct x G matmuls after the final GRU step
            for k in range(K):
                for hc in range(2):
                    psg = psp.tile([128, B], DT.float32, tag="ps",
                                   name=f"g{k}{hc}")
                    for dc in range(4):
                        nc.tensor.matmul(
                            psg[:], wkT2[:, dc, k, hc * 128:(hc + 1) * 128],
                            ztk[:, dc, k, :],
                            start=(dc == 0), stop=(dc == 3))
                    nc.vector.tensor_copy(GT[:, hc, k, :], psg[:])

        gru3_prep = {}

        def emit_gru3_prep():
            # step-3 hidden-state matmuls + bias merges, emitted a row early
            # (hT(2) is ready) so only the gi-dependent ops remain at the end
            gh_rz = psp.tile([NB, 2 * HID], DT.float32, tag="ps", name="ghrz3")
            gh_n = psp.tile([NB, HID], DT.float32, tag="ps", name="ghn3")
            for c in range(2):
                nc.tensor.matmul(gh_rz[:], hT[:, c, :], ghhT[:, c, :2 * HID],
                                 start=(c == 0), stop=(c == 1))
            for c in range(2):
                nc.tensor.matmul(gh_n[:], hT[:, c, :], ghhT[:, c, 2 * HID:],
                                 start=(c == 0), stop=(c == 1))
            rzh = sp.tile([NB, 2 * HID], DT.float32, tag="rzh")
            tmp0 = sp.tile([NB, HID], DT.float32, tag="tmp0")
            nc.vector.tensor_add(rzh[:], gh_rz[:], gbc[:])
            nc.vector.tensor_add(tmp0[:], gh_n[:], gbhh[:, 2 * HID:])
            gru3_prep['rzh'] = rzh
            gru3_prep['tmp0'] = tmp0

        def run_hooks(s):
            for fn in hooks[s]:
                fn()
            hooks[s] = []

        # flatten slots so each slot can prefetch the next slot's conv1
        slot_list = []
        for t in ROWS:
            slot_list.append((t * 4 + 0, t * 4 + 1, 0, False))
            slot_list.append((t * 4 + 2, t * 4 + 3, 1, t == 3))
        slot_states = [({}, {}) for _ in slot_list]
        emit_conv1(slot_list[0][0], slot_states[0][0])
        emit_conv1(slot_list[0][1], slot_states[0][1])

        def emit_row_slots(t):
            for s in (0, 1):
                idx = ROWS.index(t) * 2 + s
                pA, pB, hs, last = slot_list[idx]
                nxt = None
                if idx + 1 < len(slot_list):
                    n = slot_list[idx + 1]
                    nxt = (n[0], n[1],
                           slot_states[idx + 1][0], slot_states[idx + 1][1])
                emit_slot(pA, pB, slot_states[idx][0], slot_states[idx][1],
                          nxt=nxt,
                          hook=(lambda s=hs: run_hooks(s)) if hooks[hs] else None,
                          last=last, nxt_early=(idx == 0))

        for t in ROWS:
            emit_row_slots(t)
            if t == 8:
                # ztk rows complete -> AllGather (gpsimd queue, overlaps conv)
                nc.gpsimd.dma_start(zin_b.ap(), zbuf[:, :, TCTX * 8:])
                nc.gpsimd.collective_compute(
                    "AllGather", ALU.bypass,
                    replica_groups=[list(range(NCORES))],
                    ins=[zin_b.ap().opt()], outs=[zout_b.ap().opt()])
                for core in range(NCORES):
                    nc.gpsimd.dma_start(
                        ztk[:, :, :, core * NB:(core + 1) * NB], zout_b[core])
            if t < 3:
                hooks[0].append(lambda t=t: emit_gru_step(t))
                hooks[1].append(lambda t=t: emit_transposes(t))
                if t == 1:
                    # gathered ztk lands mid row 1 -> compute G during row 2
                    hooks[0].append(emit_G)
                if t == 2:
                    hooks[1].append(emit_gru3_prep)

        # ---------------- endgame: GRU step 3 + raw scores ----------------
        # (host does the double-exp log-softmax in float64 from raw scores)
        gi_rz3 = psp.tile([NB, 2 * HID], DT.float32, tag="ps", name="girz3")
        gi_n3 = psp.tile([NB, HID], DT.float32, tag="ps", name="gin3")
        for c in range(4):
            nc.tensor.matmul(gi_rz3[:], zbuf[:, c, 24:32],
                             gihT[:, c, :2 * HID], start=(c == 0), stop=(c == 3))
        for c in range(4):
            nc.tensor.matmul(gi_n3[:], zbuf[:, c, 24:32],
                             gihT[:, c, 2 * HID:], start=(c == 0), stop=(c == 3))
        # single-bank score accumulator: bias + ct matmuls write disjoint
        # 64-col slices of one [8, 320] PSUM tile as one accumulation group
        # (per-element has_written bits make the slices independent)
        psk = psp.tile([NB, K * B], DT.float32, tag="ps", name="sck")
        for k in range(K):
            for dc in range(4):
                nc.tensor.matmul(psk[:, k * B:(k + 1) * B],
                                 wkbrep[:, dc, k, :], ztk[:, dc, k, :],
                                 start=(k == 0 and dc == 0), stop=False)
        # GRU-3 chain with the gh/bias parts pre-merged (rzh, tmp0); r/z and
        # tanh/finale split so each consumer unblocks as early as possible
        rzh, tmp0 = gru3_prep['rzh'], gru3_prep['tmp0']
        rz = sp.tile([NB, 2 * HID], DT.float32, tag="rz", name="rz3")
        ng = sp.tile([NB, HID], DT.float32, tag="ng", name="ng3")
        tmpf = sp.tile([NB, HID], DT.float32, tag="gtmp", name="gtmp3")
        nc.vector.tensor_add(rz[:, :HID], gi_rz3[:, :HID], rzh[:, :HID])
        nc.vector.tensor_add(ng[:], gi_n3[:], gbih[:, 2 * HID:])
        nc.vector.tensor_add(rz[:, HID:], gi_rz3[:, HID:], rzh[:, HID:])
        nc.scalar.activation(rz[:, :HID], rz[:, :HID], ACTF.Sigmoid)
        nc.scalar.activation(rz[:, HID:], rz[:, HID:], ACTF.Sigmoid)
        nc.vector.tensor_mul(tmpf[:], tmp0[:], rz[:, :HID])
        nc.vector.tensor_add(ng[:], ng[:], tmpf[:])
        h_prev = gru_state['h']
        h_new = sp.tile([NB, HID], DT.float32, tag="hstep3", name="hnew3")
        for c in range(2):
            cs = slice(c * 128, (c + 1) * 128)
            nc.scalar.activation(ng[:, cs], ng[:, cs], ACTF.Tanh)
            nc.vector.tensor_sub(tmpf[:, cs], h_prev[:, cs], ng[:, cs])
            nc.vector.tensor_mul(tmpf[:, cs],
                                 rz[:, HID + c * 128:HID + (c + 1) * 128],
                                 tmpf[:, cs])
            nc.vector.tensor_add(h_new[:, cs], ng[:, cs], tmpf[:, cs])
            pt = psp.tile([128, NB], DT.float32, tag="ps", name=f"ptf{c}")
            nc.tensor.transpose(pt[:], h_new[:, cs], ident[:])
            nc.vector.tensor_copy(hT[:, c, :], pt[:])
        for hc in range(2):
            for k in range(K):
                nc.tensor.matmul(psk[:, k * B:(k + 1) * B],
                                 hT[:, hc, :], GT[:, hc, k, :],
                                 start=False, stop=(hc == 1 and k == K - 1))
        nc.scalar.activation(y40[:], psk[:], ACTF.Identity)
        nc.sync.dma_start(out_d[:], y40[:])

    nc.compile()
    return nc


def host_prep(inputs):
    """Host-side prep: im2col for conv1, weight layout transforms, bf16 casts."""
    x = np.asarray(inputs['x'], F32)
    xp = np.pad(x, ((0, 0), (0, 0), (0, 0), (2, 2), (2, 2)))
    s = xp.strides
    xs = np.lib.stride_tricks.as_strided(
        xp, shape=(B, T, C, 5, 5, 16, 16),
        strides=(s[0], s[1], s[2], s[3], s[4], 2 * s[3], 2 * s[4]))
    x_col = np.ascontiguousarray(xs).reshape(B, T, 75, NPIX).astype(BF16)

    xcols = []
    for core in range(NCORES):
        xc = x_col[core * NB:(core + 1) * NB]
        arr = np.zeros((NPAIR, 128, 2 * NPIX), BF16)
        for t in range(T):
            for j in range(NB // 2):
                p = t * 4 + j
                arr[p, :75, :NPIX] = xc[2 * j, t]
                arr[p, :75, NPIX:] = xc[2 * j + 1, t]
        xcols.append(arr)

    w = {}
    w1T = np.zeros((128, DIM), BF16)
    w1T[:75] = np.asarray(inputs['enc_w'], F32).reshape(DIM, 75).T.astype(BF16)
    w['w1T'] = w1T
    r1 = np.asarray(inputs['res_w1'], F32).reshape(R, HALF, DIM).transpose(0, 2, 1)
    w['r1T'] = np.ascontiguousarray(
        r1.reshape(R, 4, 128, HALF).transpose(2, 0, 1, 3)).astype(BF16)
    # fp8 DoubleRow weights for resblock 0 (x64, undone by drain scale 2^-6)
    w['r1T8'] = np.ascontiguousarray(
        (r1[0] * 64.0).reshape(4, 128, 2, 128).transpose(1, 0, 2, 3)
        .reshape(128, 2, 2, 2, 128)).astype(E4M3)
    r2 = np.asarray(inputs['res_w2'], F32).transpose(0, 3, 4, 2, 1)
    w['w2T'] = np.ascontiguousarray(
        r2.reshape(R, 9, 2, 128, HALF).transpose(3, 0, 1, 2, 4)).astype(BF16)
    w['w2T8'] = np.ascontiguousarray(
        (r2[0] * 64.0).reshape(9, 2, 128, 2, 128)
        .transpose(2, 0, 1, 3, 4)).astype(E4M3)
    r3 = np.asarray(inputs['res_w3'], F32).reshape(R, DIM, HALF).transpose(0, 2, 1)
    w['r3T'] = np.ascontiguousarray(
        r3.reshape(R, 2, 128, DIM).transpose(2, 0, 1, 3)).astype(BF16)
    w['r3T8'] = np.ascontiguousarray(
        (r3[0] * 64.0).reshape(2, 128, 4, 128).transpose(1, 0, 2, 3)).astype(E4M3)
    w['encb'] = np.ascontiguousarray(
        np.asarray(inputs['enc_b'], F32).reshape(4, 128).T)
    w['b1'] = np.ascontiguousarray(
        np.asarray(inputs['res_b1'], F32).reshape(R, 2, 128).transpose(2, 0, 1))
    w['b2'] = np.ascontiguousarray(
        np.asarray(inputs['res_b2'], F32).reshape(R, 2, 128).transpose(2, 0, 1))
    w['b3'] = np.ascontiguousarray(
        np.asarray(inputs['res_b3'], F32).reshape(R, 4, 128).transpose(2, 0, 1))
    w['gihT'] = np.ascontiguousarray(
        np.asarray(inputs['gru_w_ih'], F32).T.reshape(4, 128, 3 * HID)
        .transpose(1, 0, 2)).astype(BF16)
    w['ghhT'] = np.ascontiguousarray(
        np.asarray(inputs['gru_w_hh'], F32).T.reshape(2, 128, 3 * HID)
        .transpose(1, 0, 2)).astype(BF16)
    bih = np.asarray(inputs['gru_b_ih'], F32)
    bhh = np.asarray(inputs['gru_b_hh'], F32)
    w['gbih'] = np.tile(bih[None, :], (NB, 1))
    w['gbhh'] = np.tile(bhh[None, :], (NB, 1))
    w['gbc'] = np.tile((bih + bhh)[None, :2 * HID], (NB, 1))
    # wkT2[d%128, d//128, k, h] = wk_w[k, d, h]  (for G = ztk . W_k)
    w['wkT2'] = np.ascontiguousarray(
        np.asarray(inputs['wk_w'], F32).reshape(K, 4, 128, HID)
        .transpose(2, 1, 0, 3)).astype(BF16)
    # wkbrep[d%128, d//128, k, i] = wk_b[k, d] (bias via matmul, repl. over i)
    wkb = np.asarray(inputs['wk_b'], F32).reshape(K, 4, 128).transpose(2, 1, 0)
    w['wkbrep'] = np.ascontiguousarray(
        np.repeat(wkb[:, :, :, None], NB, axis=3)).astype(BF16)
    w['ident8'] = np.eye(NB, dtype=F32)
    return xcols, w


_NC_CACHE = {}


def get_nc():
    if 'nc' not in _NC_CACHE:
        _NC_CACHE['nc'] = build_kernel()
    return _NC_CACHE['nc']


def make_in_maps(inputs):
    xcols, w = host_prep(inputs)
    in_maps = []
    for core in range(NCORES):
        m = dict(w)
        m['xcol'] = xcols[core]
        in_maps.append(m)
    return in_maps


def reduce_outputs(results):
    # raw scores s[i, b, k]: anchor i (this core's batch rows), candidate b
    s = np.empty((B, B, K), np.float64)
    for core in range(NCORES):
        o = np.asarray(results[core]['out'], F32)   # [8, K*B]
        s[core * NB:(core + 1) * NB] = (
            o.reshape(NB, K, B).transpose(0, 2, 1))
    # reference: lsm = log_softmax(exp(s), axis=1) over candidates b
    y = np.exp(s)
    m = y.max(axis=1, keepdims=True)
    lse = np.log(np.exp(y - m).sum(axis=1, keepdims=True)) + m
    lsm = y - lse
    idx = np.arange(B)
    loss = np.float32(-lsm[idx, idx, :].mean())
    acc = np.float32((lsm.argmax(axis=1) == idx[:, None]).mean())
    return loss, acc


def _install_ntff_hook():
    """Provide antenv.axon_hooks (missing in this image) so trace=True works."""
    try:
        from antenv.axon_hooks import get_axon_ntff_profile_hook  # noqa: F401
        return
    except ImportError:
        pass
    import ctypes
    import types
    import contextlib

    so_path = "/opt/axon/libaxon_pjrt.so"
    if not os.path.exists(so_path):
        return
    lib = ctypes.CDLL(so_path)
    if not hasattr(lib, "axon_start_nrt_profile"):
        return
    lib.axon_start_nrt_profile.argtypes = [ctypes.POINTER(ctypes.c_int64),
                                           ctypes.c_size_t]
    lib.axon_start_nrt_profile.restype = ctypes.c_int64
    lib.axon_stop_nrt_profile.argtypes = [ctypes.c_char_p]
    lib.axon_stop_nrt_profile.restype = ctypes.c_int64

    @contextlib.contextmanager
    def _hook(output_dir, device_ids):
        import jax
        jax.devices()
        if device_ids:
            ids = (ctypes.c_int64 * len(device_ids))(*device_ids)
            rc = lib.axon_start_nrt_profile(ids, len(device_ids))
        else:
            rc = lib.axon_start_nrt_profile(None, 0)
        if rc != 0:
            raise RuntimeError(f"axon_start_nrt_profile rc={rc}")
        try:
            yield
        finally:
            n = lib.axon_stop_nrt_profile(str(output_dir).encode())
            print(f"ntff profile: {n} file(s) written to {output_dir}")

    mod = types.ModuleType("antenv.axon_hooks")
    mod.get_axon_ntff_profile_hook = lambda: _hook
    mod.set_axon_ntff_profile_hook = lambda h: None
    import antenv
    antenv.axon_hooks = mod
    sys.modules["antenv.axon_hooks"] = mod


def run(inputs, trace=False, **kw):
    if trace:
        _install_ntff_hook()
    nc = get_nc()
    in_maps = make_in_maps(inputs)
    res = run_bass_kernel_spmd(nc, in_maps, core_ids=list(range(NCORES)),
                               trace=trace, **kw)
    return res


def kernel(**inputs):
    res = run(inputs, trace=False)
    return reduce_outputs(res.results)


if __name__ == '__main__':
    import reference as Rf
    inputs = {k: np.asarray(v) for k, v in Rf.setup_inputs().items()}
    loss, acc = kernel(**inputs)
    print('kernel loss/acc:', loss, acc)

